# revision 21
# baseline (speedup 1.0000x reference)
"""GAT (2-layer GATConv + FF head) on 8 Trainium2 NeuronCores.

Strategy (per sharding hint): nodes + incident edges partitioned by
destination across 8 cores; per-edge softmax/scatter local to the
destination shard via one-hot matmul-scatter into PSUM; small weights
replicated. Layer-1 node features are computed fully replicated (input x
is available everywhere); layer-2 features are computed on the owning
shard and exchanged with a single feature AllGather. Random-access reads
(h[src] rows, own-shard a_dst rows) use SWDGE dma_gather from fp16 DRAM
tables. Per-edge a_src is recomputed on the fly from the gathered rows
(dot with att_src on VectorE), so no global narrow tables or narrow
exchange exist. The layer-2 dense is fused into the layer-1 edge-phase
evict so t2h_own completes with the edge phase and the AllGather fires
immediately; each tile's local nd gather + one-hot build are issued
before its table gathers so local work overlaps the collective.

Message path in fp16 (tables, gathered rows, attention weights, matmul
operands); accumulation in fp32 PSUM; per-edge logits in fp32.
"""
import sys
sys.path.insert(0, "/opt/trn_rl_repo")

import numpy as np
from contextlib import ExitStack

import concourse.bass as bass
import concourse.bacc as bacc
import concourse.tile as tile
import concourse.mybir as mybir
from concourse.bass_utils import run_bass_kernel_spmd

dt = mybir.dt
OP = mybir.AluOpType
ACT = mybir.ActivationFunctionType

NCORES = 8
H = 4
NEG_SLOPE = 0.2


# ----------------------------------------------------------------------------
# host-side prep
# ----------------------------------------------------------------------------

def _wrap_idx(idx):
    """Pack an index list into the SWDGE wrapped layout [128, n/16] int16:
    index i -> partition i%16 (replicated to all 8 16-partition groups),
    free offset i//16."""
    n = len(idx)
    assert n % 128 == 0
    out = np.zeros((128, n // 16), np.int16)
    a = np.asarray(idx, np.int16).reshape(n // 16, 16).T  # [16, n/16]
    for r in range(8):
        out[r * 16:(r + 1) * 16, :] = a
    return out


def _pad128(a, fill):
    n = len(a)
    m = ((n + 127) // 128) * 128
    return np.concatenate([a, np.full(m - n, fill, a.dtype)])


class Sched:
    """Static, core-uniform per-tile chunk schedule."""

    def __init__(self, n_lo, n_hi):
        self.n_lo = n_lo          # [NT] chunks for lo-half gathers
        self.n_hi = n_hi          # [NT] chunks for hi-half gathers
        self.ct = [a + b for a, b in zip(n_lo, n_hi)]
        self.base = np.concatenate([[0], np.cumsum(self.ct)]).astype(int)
        self.total = int(self.base[-1])  # total chunks per core


_EDGE_CACHE = {}


def _prep(x, edge_index, W1, att_src1, att_dst1, b1, W2, att_src2, att_dst2,
          b2, ff1_w, ff1_b, ff2_w, ff2_b):
    N, IN = x.shape
    F = W1.shape[1]               # 256
    C1 = F // H
    C2 = W2.shape[1] // H
    NSH = N // NCORES
    NT = (NSH + 127) // 128
    NLO_T = (N // 2) // 128
    SPLIT = NLO_T * 128

    import hashlib as _hl
    ekey = (N, _hl.sha1(np.ascontiguousarray(edge_index).tobytes())
            .hexdigest())
    cached = _EDGE_CACHE.get(ekey)
    if cached is not None:
        (sched, idx_h, idx_nd, dstloc, idx_own_lo, idx_own_hi,
         own_mask) = cached
        return _prep_finish(x, W1, att_src1, att_dst1, b1, W2, att_src2,
                            att_dst2, b2, ff1_w, ff1_b, ff2_w, ff2_b,
                            sched, idx_h, idx_nd, dstloc, idx_own_lo,
                            idx_own_hi, own_mask,
                            N, IN, F, NSH, NT, SPLIT, C1, C2)

    E = edge_index.shape[1]
    ar = np.arange(N, dtype=np.int64)
    src = np.concatenate([edge_index[0], ar])
    dst = np.concatenate([edge_index[1], ar])

    shard = dst // NSH
    dstloc_all = dst - shard * NSH

    # group edges per (core, tile, half)
    per = [[[None, None] for _ in range(NT)] for _ in range(NCORES)]
    for k in range(NCORES):
        m = shard == k
        s_k, dl_k = src[m], dstloc_all[m]
        t_k = dl_k // 128
        for t in range(NT):
            mt = t_k == t
            s_t, dl_t = s_k[mt], dl_k[mt]
            lo = s_t < SPLIT
            per[k][t][0] = (s_t[lo], dl_t[lo])
            per[k][t][1] = (s_t[~lo] - SPLIT, dl_t[~lo])

    n_lo = [max((len(per[k][t][0][0]) + 127) // 128 for k in range(NCORES))
            for t in range(NT)]
    n_hi = [max((len(per[k][t][1][0]) + 127) // 128 for k in range(NCORES))
            for t in range(NT)]
    sched = Sched(n_lo, n_hi)

    # per-core edge arrays in schedule order
    idx_h = []       # [128, total*8] int16  (gather idx, lo/hi-local rows)
    idx_nd = []      # [128, total*8] int16  (dstlocal shard rows)
    dstloc = []      # [128, total] f32      (tile-local dst or -1)
    for k in range(NCORES):
        ih = np.zeros((128, sched.total * 8), np.int16)
        nd = np.zeros((128, sched.total * 8), np.int16)
        dl = np.full((128, sched.total), -1.0, np.float32)
        for t in range(NT):
            off = sched.base[t]
            for half, nch in ((0, n_lo[t]), (1, n_hi[t])):
                if nch == 0:
                    continue
                s_t, dl_t = per[k][t][half]
                ne = nch * 128
                sp = _pad128(np.concatenate([s_t, np.zeros(ne - len(s_t),
                                                           np.int64)]), 0)[:ne]
                sp[len(s_t):] = 0
                dlp = np.full(ne, -1.0, np.float32)
                dlp[:len(dl_t)] = (dl_t - t * 128).astype(np.float32)
                ndp = np.zeros(ne, np.int64)
                ndp[:len(dl_t)] = dl_t
                ih[:, off * 8:(off + nch) * 8] = _wrap_idx(sp)
                nd[:, off * 8:(off + nch) * 8] = _wrap_idx(ndp)
                dl[:, off:off + nch] = dlp.reshape(nch, 128).T
                off += nch
        idx_h.append(ih)
        idx_nd.append(nd)
        dstloc.append(dl)

    # own-narrow build: gather T1n rows for own shard (lo/hi + select mask)
    own_rows_pad = NT * 128
    idx_own_lo, idx_own_hi, own_mask = [], [], []
    for k in range(NCORES):
        rows = np.arange(k * NSH, (k + 1) * NSH)
        rows = np.concatenate([rows, np.full(own_rows_pad - NSH, rows[0])])
        is_lo = rows < SPLIT
        lo_i = np.where(is_lo, rows, 0)
        hi_i = np.where(is_lo, 0, rows - SPLIT)
        idx_own_lo.append(_wrap_idx(lo_i))
        idx_own_hi.append(_wrap_idx(hi_i))
        m = np.zeros((128, NT), np.float16)
        m[:, :] = is_lo.reshape(NT, 128).T.astype(np.float16)
        own_mask.append(m)

    _EDGE_CACHE[ekey] = (sched, idx_h, idx_nd, dstloc, idx_own_lo,
                         idx_own_hi, own_mask)
    return _prep_finish(x, W1, att_src1, att_dst1, b1, W2, att_src2,
                        att_dst2, b2, ff1_w, ff1_b, ff2_w, ff2_b,
                        sched, idx_h, idx_nd, dstloc, idx_own_lo,
                        idx_own_hi, own_mask,
                        N, IN, F, NSH, NT, SPLIT, C1, C2)


def _prep_finish(x, W1, att_src1, att_dst1, b1, W2, att_src2, att_dst2, b2,
                 ff1_w, ff1_b, ff2_w, ff2_b, sched, idx_h, idx_nd, dstloc,
                 idx_own_lo, idx_own_hi, own_mask,
                 N, IN, F, NSH, NT, SPLIT, C1, C2):
    # weights
    def aug(W, a_s, a_d, C):
        v_s = np.einsum("fhc,hc->fh", W.reshape(-1, H, C), a_s)
        v_d = np.einsum("fhc,hc->fh", W.reshape(-1, H, C), a_d)
        return np.concatenate([W, v_s, v_d], axis=1).astype(np.float16)

    W1aug = aug(W1, att_src1, att_dst1, C1)              # [IN, F+8]
    W2aug = aug(W2, att_src2, att_dst2, C2)              # [F, F+8]
    W2aug_pk = W2aug.reshape(2, 128, F + 8).transpose(1, 0, 2).copy()

    xT16 = np.ascontiguousarray(x.T).astype(np.float16)  # [IN, N]

    iota16 = np.tile(np.arange(128, dtype=np.float16), (128, 1))
    ident16 = np.eye(128, dtype=np.float16)

    const = {
        "xT16": xT16, "W1aug": W1aug, "W2aug": W2aug_pk,
        "iota16": iota16, "ident16": ident16,
        "a1srep": np.tile(att_src1.astype(np.float16).reshape(1, F),
                          (128, 1)),
        "a2srep": np.tile(att_src2.astype(np.float16).reshape(1, F),
                          (128, 1)),
        "b1rep": np.tile(b1.astype(np.float32), (128, 1)),
        "b2rep": np.tile(b2.astype(np.float32), (128, 1)),
        "f1brep": np.tile(ff1_b.astype(np.float32), (128, 1)),
        "f2brep": np.tile(ff2_b.astype(np.float32), (128, 1)),
        "ff1w16": ff1_w.astype(np.float16),
        "ff2w16": ff2_w.astype(np.float16),
    }

    in_maps = []
    for k in range(NCORES):
        m = dict(const)
        m.update({
            "idx_h": idx_h[k], "idx_nd": idx_nd[k], "dstloc": dstloc[k],
            "idx_own_lo": idx_own_lo[k], "idx_own_hi": idx_own_hi[k],
            "own_mask": own_mask[k],
        })
        in_maps.append(m)

    dims = dict(N=N, IN=IN, F=F, NSH=NSH, NT=NT, SPLIT=SPLIT,
                NLO=SPLIT, NHI=N - SPLIT, C2=C2, FH=ff1_w.shape[1])
    return in_maps, sched, dims


# ----------------------------------------------------------------------------
# device program
# ----------------------------------------------------------------------------

def _gather_split(nc, out_ap_fn, tab, idx_sb, n_chunks, elem, q0):
    """Emit dma_gather calls capped at 8 chunks (1024 idxs) each.
    out_ap_fn(c0, c1) -> output AP for chunk range; idx_sb indexed per chunk."""
    c0 = 0
    q = q0
    while c0 < n_chunks:
        c1 = min(c0 + 8, n_chunks)
        nc.gpsimd.dma_gather(
            out_ap_fn(c0, c1), tab, idx_sb[:, c0 * 8:c1 * 8],
            num_idxs=(c1 - c0) * 128, num_idxs_reg=(c1 - c0) * 128,
            elem_size=elem, queue_num=q % 4)
        q += 1
        c0 = c1


def _build(sched, dims):
    import os, hashlib
    PH = int(os.environ.get('K_PHASES', '6'))
    TAPS = int(os.environ.get('K_TAPS', '0'))
    N, IN, F, NSH, NT = dims["N"], dims["IN"], dims["F"], dims["NSH"], dims["NT"]
    NLO, NHI, SPLIT = dims["NLO"], dims["NHI"], dims["SPLIT"]
    C2, FH = dims["C2"], dims["FH"]
    FA = F + 8
    NTG = (N + 127) // 128          # global node tiles
    NTG_LO = NLO // 128

    nc = bacc.Bacc("TRN2", target_bir_lowering=False, num_devices=NCORES,
                   num_swdge_queues=4)
    # The neuronx compile cache keys on the jit signature only (the embedded
    # BIR is not hashed), so two different programs with identical I/O would
    # alias to one NEFF. A content-named dummy input de-aliases them.
    with open(__file__, "rb") as _f:
        _salt = hashlib.sha256(
            _f.read() + repr((sched.n_lo, sched.n_hi, sorted(dims.items()),
                              PH)).encode()).hexdigest()[:16]
    nc.dram_tensor(f"salt_{_salt}", [1, 4], dt.float32, kind="ExternalInput")
    dims["salt_name"] = f"salt_{_salt}"

    # inputs
    xT16 = nc.dram_tensor("xT16", [IN, N], dt.float16, kind="ExternalInput")
    W1aug = nc.dram_tensor("W1aug", [IN, FA], dt.float16, kind="ExternalInput")
    W2aug = nc.dram_tensor("W2aug", [128, 2, FA], dt.float16, kind="ExternalInput")
    iota_in = nc.dram_tensor("iota16", [128, 128], dt.float16, kind="ExternalInput")
    ident_in = nc.dram_tensor("ident16", [128, 128], dt.float16, kind="ExternalInput")
    b1rep = nc.dram_tensor("b1rep", [128, F], dt.float32, kind="ExternalInput")
    b2rep = nc.dram_tensor("b2rep", [128, C2], dt.float32, kind="ExternalInput")
    f1brep = nc.dram_tensor("f1brep", [128, FH], dt.float32, kind="ExternalInput")
    f2brep = nc.dram_tensor("f2brep", [128, 2], dt.float32, kind="ExternalInput")
    ff1w16 = nc.dram_tensor("ff1w16", [C2, FH], dt.float16, kind="ExternalInput")
    ff2w16 = nc.dram_tensor("ff2w16", [FH, 2], dt.float16, kind="ExternalInput")
    a1srep_d = nc.dram_tensor("a1srep", [128, F], dt.float16, kind="ExternalInput")
    a2srep_d = nc.dram_tensor("a2srep", [128, F], dt.float16, kind="ExternalInput")
    idx_h_d = nc.dram_tensor("idx_h", [128, sched.total * 8], dt.int16, kind="ExternalInput")
    idx_nd_d = nc.dram_tensor("idx_nd", [128, sched.total * 8], dt.int16, kind="ExternalInput")
    dstloc_d = nc.dram_tensor("dstloc", [128, sched.total], dt.float32, kind="ExternalInput")
    iol_d = nc.dram_tensor("idx_own_lo", [128, NT * 8], dt.int16, kind="ExternalInput")
    ioh_d = nc.dram_tensor("idx_own_hi", [128, NT * 8], dt.int16, kind="ExternalInput")
    omask_d = nc.dram_tensor("own_mask", [128, NT], dt.float16, kind="ExternalInput")

    out_d = nc.dram_tensor("out", [NSH, 2], dt.float32, kind="ExternalOutput")
    if TAPS:
        tap_h = nc.dram_tensor("tap_h", [256, F], dt.float16, kind="ExternalOutput")
        tap_n = nc.dram_tensor("tap_n", [256, 128], dt.float16, kind="ExternalOutput")
        tap_own = nc.dram_tensor("tap_own", [256, 128], dt.float16, kind="ExternalOutput")
        tap_h1 = nc.dram_tensor("tap_h1", [NSH, F], dt.float16, kind="ExternalOutput")
        tap_t2 = nc.dram_tensor("tap_t2", [256, F], dt.float16, kind="ExternalOutput")
        tap_ag = nc.dram_tensor("tap_ag", [256, F], dt.float16, kind="ExternalOutput")
        tap_h2 = nc.dram_tensor("tap_h2", [NSH, C2], dt.float16, kind="ExternalOutput")
        tap_f1 = nc.dram_tensor("tap_f1", [NSH, FH], dt.float16, kind="ExternalOutput")

    with tile.TileContext(nc) as tc, ExitStack() as octx:
        # persistent pools
        dram = octx.enter_context(tc.tile_pool(name="dram", bufs=1, space="DRAM"))
        cpool = octx.enter_context(tc.tile_pool(name="const", bufs=1))
        stash = octx.enter_context(tc.tile_pool(name="stash", bufs=1))

        # DRAM tables
        t1h_lo = dram.tile([NLO, F], dt.float16)
        t1h_hi = dram.tile([NHI, F], dt.float16)
        t1n_lo = dram.tile([NLO, 128], dt.float16)
        t1n_hi = dram.tile([NHI, 128], dt.float16)
        t1n_own = dram.tile([NT * 128, 128], dt.float16)
        t2h_own = dram.tile([NSH, F], dt.float16)
        t2n_own = dram.tile([NT * 128, 128], dt.float16)
        t2h_all = dram.tile([N, F], dt.float16, addr_space="Shared")

        # constants in SBUF
        iota16 = cpool.tile([128, 128], dt.float16)
        nc.sync.dma_start(iota16[:], iota_in[:])
        ident16 = cpool.tile([128, 128], dt.float16)
        nc.sync.dma_start(ident16[:], ident_in[:])
        w1a_sb = cpool.tile([IN, FA], dt.float16)
        nc.sync.dma_start(w1a_sb[:], W1aug[:])
        w2a_sb = cpool.tile([128, 2, FA], dt.float16)
        nc.sync.dma_start(w2a_sb[:], W2aug[:])
        b1_sb = cpool.tile([128, F], dt.float32)
        nc.sync.dma_start(b1_sb[:], b1rep[:])
        b2_sb = cpool.tile([128, C2], dt.float32)
        nc.sync.dma_start(b2_sb[:], b2rep[:])
        f1b_sb = cpool.tile([128, FH], dt.float32)
        nc.sync.dma_start(f1b_sb[:], f1brep[:])
        f2b_sb = cpool.tile([128, 2], dt.float32)
        nc.sync.dma_start(f2b_sb[:], f2brep[:])
        ff1_sb = cpool.tile([C2, FH], dt.float16)
        nc.sync.dma_start(ff1_sb[:], ff1w16[:])
        ff2_sb = cpool.tile([FH, 2], dt.float16)
        nc.sync.dma_start(ff2_sb[:], ff2w16[:])
        a1s_sb = cpool.tile([128, F], dt.float16)
        nc.sync.dma_start(a1s_sb[:], a1srep_d[:])
        a2s_sb = cpool.tile([128, F], dt.float16)
        nc.sync.dma_start(a2s_sb[:], a2srep_d[:])

        # layer-1 hidden transposed, kept in SBUF for the layer-2 dense
        h1T = stash.tile([128, 2, NT, 128], dt.float16)
        out_stage = stash.tile([128, NT, 2], dt.float32)

        # ------------------------------------------------------------------
        # phase A: full replicated layer-1 dense -> T1 tables
        # ------------------------------------------------------------------
        with ExitStack() as ctx:
            xp = ctx.enter_context(tc.tile_pool(name="xp", bufs=2))
            pp = ctx.enter_context(tc.tile_pool(name="pp", bufs=4, space="PSUM"))
            sp = ctx.enter_context(tc.tile_pool(name="sp", bufs=2))

            G = 8
            groups = []
            m0 = 0
            while m0 < NTG:
                g = min(G, NTG - m0)
                groups.append((m0, g))
                m0 += g
            for (m0, g) in groups:
                xs = xp.tile([IN, G * 128], dt.float16, tag="xs")
                rows_t = min(g * 128, N - m0 * 128)
                nc.sync.dma_start(xs[:, 0:rows_t], xT16[:, m0 * 128:m0 * 128 + rows_t])
                hst = sp.tile([128, G, F], dt.float16, tag="hst")
                nst = sp.tile([128, G, 128], dt.float16, tag="nst")
                for j in range(g):
                    m = m0 + j
                    rows = min(128, N - m * 128)
                    ps = pp.tile([128, FA], dt.float32, tag="ps")
                    nc.tensor.matmul(ps[0:rows, :], xs[:, j * 128:j * 128 + rows],
                                     w1a_sb[:], start=True, stop=True)
                    nc.scalar.activation(hst[0:rows, j, :], ps[0:rows, 0:F], ACT.Copy)
                    nc.vector.tensor_copy(nst[0:rows, j, 0:8], ps[0:rows, F:FA])
                # group DMA to tables (handle a group straddling SPLIT)
                r0 = m0 * 128
                rows_t = min(g * 128, N - r0)
                if r0 + rows_t <= SPLIT:
                    dst_h, dst_n, rr = t1h_lo, t1n_lo, r0
                elif r0 >= SPLIT:
                    dst_h, dst_n, rr = t1h_hi, t1n_hi, r0 - SPLIT
                else:
                    dst_h = None
                if dst_h is not None:
                    _wr_rows(nc, dst_h, rr, rows_t, hst, F)
                    _wr_rows(nc, dst_n, rr, rows_t, nst, 128)
                else:
                    a = SPLIT - r0
                    _wr_rows(nc, t1h_lo, r0, a, hst, F)
                    _wr_rows(nc, t1n_lo, r0, a, nst, 128)
                    _wr_rows(nc, t1h_hi, 0, rows_t - a, hst, F, col0=a // 128)
                    _wr_rows(nc, t1n_hi, 0, rows_t - a, nst, 128, col0=a // 128)

        # ------------------------------------------------------------------
        # own-narrow table for layer 1 (gather own rows from t1n lo/hi)
        # ------------------------------------------------------------------
        if PH >= 2:
         with ExitStack() as ctx:
            op_ = ctx.enter_context(tc.tile_pool(name="op", bufs=1))
            il = op_.tile([128, NT * 8], dt.int16)
            nc.sync.dma_start(il[:], iol_d[:])
            ih = op_.tile([128, NT * 8], dt.int16)
            nc.sync.dma_start(ih[:], ioh_d[:])
            msk = op_.tile([128, NT], dt.float16)
            nc.sync.dma_start(msk[:], omask_d[:])
            glo = op_.tile([128, NT, 128], dt.float16)
            _gather_split(nc, lambda a, b: glo[:, a:b, :], t1n_lo[:], il, NT,
                          128, 0)
            ghi = op_.tile([128, NT, 128], dt.float16)
            _gather_split(nc, lambda a, b: ghi[:, a:b, :], t1n_hi[:], ih, NT,
                          128, 1)
            mrg = op_.tile([128, NT, 128], dt.float16)
            # mrg = ghi + (glo - ghi) * mask
            nc.vector.tensor_tensor(mrg[:], glo[:], ghi[:], op=OP.subtract)
            nc.vector.tensor_tensor(
                mrg[:], mrg[:],
                msk[:].unsqueeze(2).broadcast_to([128, NT, 128]), op=OP.mult)
            nc.vector.tensor_tensor(mrg[:], mrg[:], ghi[:], op=OP.add)
            nc.sync.dma_start(
                t1n_own[:].rearrange("(t p) c -> p t c", p=128), mrg[:])

        # ------------------------------------------------------------------
        # edge phases
        # ------------------------------------------------------------------
        def edge_phase(ctx, name, tab_lo, tab_hi, n_own, asrep_sb, evict):
            ep = ctx.enter_context(tc.tile_pool(name=name + "e", bufs=2))
            pp = ctx.enter_context(tc.tile_pool(name=name + "p", bufs=2, space="PSUM"))
            for t in range(NT):
                ct = sched.ct[t]
                if ct == 0:
                    continue
                nlo, nhi = sched.n_lo[t], sched.n_hi[t]
                b0 = sched.base[t]
                ixh = ep.tile([128, ct * 8], dt.int16, tag="ixh")
                nc.sync.dma_start(ixh[:], idx_h_d[:, b0 * 8:(b0 + ct) * 8])
                ixn = ep.tile([128, ct * 8], dt.int16, tag="ixn")
                nc.sync.dma_start(ixn[:], idx_nd_d[:, b0 * 8:(b0 + ct) * 8])
                dl = ep.tile([128, ct], dt.float32, tag="dl")
                nc.sync.dma_start(dl[:], dstloc_d[:, b0:b0 + ct])

                # local-table gather + one-hot build first: at the layer-2
                # boundary these depend only on phase-C outputs, so they can
                # proceed while the AllGather (which gates the g gathers
                # below) is still in flight.
                g = ep.tile([128, ct, F], dt.float16, tag="g")
                nd_ = ep.tile([128, ct, 128], dt.float16, tag="nd")
                _gather_split(nc, lambda a, b: nd_[:, a:b, :], n_own, ixn,
                              ct, 128, 1)
                oh = ep.tile([128, ct, 128], dt.float16, tag="oh")
                nc.vector.tensor_tensor(
                    oh[:],
                    iota16[:].unsqueeze(1).broadcast_to([128, ct, 128]),
                    dl[:].unsqueeze(2).broadcast_to([128, ct, 128]),
                    op=OP.is_equal)
                if nlo:
                    _gather_split(nc, lambda a, b: g[:, a:b, :], tab_lo,
                                  ixh, nlo, F, 0)
                if nhi:
                    _gather_split(
                        nc, lambda a, b: g[:, nlo + a:nlo + b, :], tab_hi,
                        ixh[:, nlo * 8:ct * 8], nhi, F, 2)

                # a_src per edge from the gathered rows: ns[e,h] =
                # sum_c g[e,h*C+c] * att_src[h,c]; rhs[:, :, 0:F] is scratch
                # for the product (overwritten later by the fold).
                rhs = ep.tile([128, ct, F + H], dt.float16, tag="rhs")
                nc.vector.tensor_tensor(
                    rhs[:, :, 0:F], g[:],
                    asrep_sb[:].unsqueeze(1).broadcast_to([128, ct, F]),
                    op=OP.mult)
                nsr = ep.tile([128, ct, H], dt.float32, tag="nsr")
                nc.vector.tensor_reduce(
                    nsr[:], rhs[:, :, 0:F].rearrange("p c (h d) -> p c h d",
                                                     h=H),
                    axis=mybir.AxisListType.X, op=OP.add)

                # narrow: alpha = lrelu(a_src + a_dst); ex = exp(alpha)
                alpha = ep.tile([128, ct, H], dt.float32, tag="alpha")
                nc.vector.tensor_tensor(alpha[:], nsr[:], nd_[:, :, H:2 * H],
                                        op=OP.add)
                nc.vector.scalar_tensor_tensor(
                    alpha[:], alpha[:], float(NEG_SLOPE), alpha[:],
                    op0=OP.mult, op1=OP.max)
                nc.scalar.activation(rhs[:, :, F:F + H], alpha[:], ACT.Exp)
                # fold: rhs[:, :, 0:F] = g * ex (per-head broadcast)
                nc.vector.tensor_tensor(
                    rhs[:, :, 0:F].rearrange("p c (h d) -> p c h d", h=H),
                    g[:].rearrange("p c (h d) -> p c h d", h=H),
                    rhs[:, :, F:F + H].unsqueeze(3).broadcast_to(
                        [128, ct, H, F // H]),
                    op=OP.mult)
                # matmul-scatter (one-hot built above, before the g gathers)
                ps = pp.tile([128, F + H], dt.float32, tag="ps")
                for c in range(ct):
                    nc.tensor.matmul(ps[:], oh[:, c, :], rhs[:, c, :],
                                     start=(c == 0), stop=(c == ct - 1))
                evict(ep, pp, t, ps)

        # ---- layer 1 evict: h1 = relu(agg/den + b1); build h1T + h1own ----
        def evict1(ep, pp, t, ps):
            rows = min(128, NSH - t * 128)
            rcp = ep.tile([128, H], dt.float32, tag="rcp")
            nc.vector.reciprocal(rcp[:], ps[:, F:F + H])
            pre = ep.tile([128, F], dt.float32, tag="pre")
            nc.vector.tensor_tensor(
                pre[:].rearrange("p (h d) -> p h d", h=H),
                ps[:, 0:F].rearrange("p (h d) -> p h d", h=H),
                rcp[:].unsqueeze(2).broadcast_to([128, H, F // H]), op=OP.mult)
            nc.vector.tensor_tensor(pre[:], pre[:], b1_sb[:], op=OP.add)
            h1r = ep.tile([128, F], dt.float16, tag="h1r")
            nc.scalar.activation(h1r[:], pre[:], ACT.Relu)
            if TAPS:
                nc.sync.dma_start(tap_h1[t * 128:t * 128 + rows, :], h1r[0:rows, :])
            for b in range(2):
                tp = pp.tile([128, 128], dt.float16, tag="tp")
                nc.tensor.transpose(tp[:], h1r[:, b * 128:(b + 1) * 128], ident16[:])
                nc.scalar.activation(h1T[:, b, t, :], tp[:], ACT.Copy)
            # fused layer-2 dense for this tile: t2h_own rows are complete as
            # soon as the L1 edge phase finishes, so the AllGather can launch
            # without a separate dense pass in between.
            ps2 = pp.tile([128, FA], dt.float32, tag="ps2")
            for b in range(2):
                nc.tensor.matmul(ps2[:], h1T[:, b, t, :], w2a_sb[:, b, :],
                                 start=(b == 0), stop=(b == 1))
            hst = ep.tile([128, F], dt.float16, tag="hst")
            nc.scalar.activation(hst[:], ps2[:, 0:F], ACT.Copy)
            nst = ep.tile([128, 128], dt.float16, tag="nst")
            nc.vector.tensor_copy(nst[:, 0:8], ps2[:, F:FA])
            nc.sync.dma_start(t2h_own[t * 128:t * 128 + rows, :],
                              hst[0:rows, :])
            nc.sync.dma_start(t2n_own[t * 128:(t + 1) * 128, :], nst[:])

        if PH >= 3:
         with ExitStack() as ctx:
            edge_phase(ctx, "l1", t1h_lo[:], t1h_hi[:], t1n_own[:], a1s_sb,
                       evict1)

        # ------------------------------------------------------------------
        # (layer-2 dense is fused into evict1 above)
        # ------------------------------------------------------------------
        # phase D: exchange (h only; per-edge a_src is recomputed from the
        # gathered rows, so no narrow-table exchange is needed)
        # ------------------------------------------------------------------
        if PH >= 5:
         nc.gpsimd.collective_compute(
            "AllGather", OP.bypass, replica_groups=[list(range(NCORES))],
            ins=[t2h_own[:].opt()], outs=[t2h_all[:].opt()])

        # ---- layer 2 evict: h2 = relu(mean_h(agg/den) + b2); FF head ----
        def evict2(ep, pp, t, ps):
            rows = min(128, NSH - t * 128)
            rcp = ep.tile([128, H], dt.float32, tag="rcp")
            nc.vector.reciprocal(rcp[:], ps[:, F:F + H])
            pre = ep.tile([128, H, C2], dt.float32, tag="pre")
            nc.vector.tensor_tensor(
                pre[:], ps[:, 0:F].rearrange("p (h d) -> p h d", h=H),
                rcp[:].unsqueeze(2).broadcast_to([128, H, C2]), op=OP.mult)
            red = ep.tile([128, C2], dt.float32, tag="red")
            nc.vector.tensor_reduce(red[:], pre[:].transpose([0, 2, 1]),
                                    axis=mybir.AxisListType.X, op=OP.add)
            nc.vector.scalar_tensor_tensor(red[:], red[:], 1.0 / H, b2_sb[:],
                                           op0=OP.mult, op1=OP.add)
            h2 = ep.tile([128, 128], dt.float16, tag="h2")
            nc.vector.memset(h2[:, C2:128], 0.0)
            nc.scalar.activation(h2[:, 0:C2], red[:], ACT.Relu)
            if TAPS:
                nc.sync.dma_start(tap_h2[t * 128:t * 128 + rows, :],
                                  h2[0:rows, 0:C2])
            # FF: out = relu(h2 @ ff1 + b1f) @ ff2 + b2f  (square transposes)
            tp = pp.tile([128, 128], dt.float16, tag="tp2", bufs=1)
            nc.tensor.transpose(tp[:], h2[:], ident16[:])
            h2T = ep.tile([C2, 128], dt.float16, tag="h2T")
            nc.scalar.activation(h2T[:], tp[0:C2, :], ACT.Copy)
            pf1 = pp.tile([128, FH], dt.float32, tag="pf1", bufs=1)
            nc.tensor.matmul(pf1[:], h2T[:], ff1_sb[:], start=True, stop=True)
            f1p = ep.tile([128, FH], dt.float32, tag="f1p")
            nc.vector.tensor_tensor(f1p[:], pf1[:], f1b_sb[:], op=OP.add)
            f1 = ep.tile([128, 128], dt.float16, tag="f1")
            nc.vector.memset(f1[:, FH:128], 0.0)
            nc.scalar.activation(f1[:, 0:FH], f1p[:], ACT.Relu)
            if TAPS:
                nc.sync.dma_start(tap_f1[t * 128:t * 128 + rows, :],
                                  f1[0:rows, 0:FH])
            tpf = pp.tile([128, 128], dt.float16, tag="tpf", bufs=1)
            nc.tensor.transpose(tpf[:], f1[:], ident16[:])
            f1T = ep.tile([FH, 128], dt.float16, tag="f1T")
            nc.scalar.activation(f1T[:], tpf[0:FH, :], ACT.Copy)
            pf2 = pp.tile([128, 2], dt.float32, tag="pf2", bufs=1)
            nc.tensor.matmul(pf2[:], f1T[:], ff2_sb[:], start=True, stop=True)
            nc.vector.tensor_tensor(out_stage[:, t, :], pf2[:], f2b_sb[:],
                                    op=OP.add)

        if TAPS:
            A = SPLIT - 128
            nc.sync.dma_start(tap_h[0:128, :], t1h_lo[A:A + 128, :])
            nc.sync.dma_start(tap_h[128:256, :], t1h_hi[0:128, :])
            nc.sync.dma_start(tap_n[0:128, :], t1n_lo[A:A + 128, :])
            nc.sync.dma_start(tap_n[128:256, :], t1n_hi[0:128, :])
            nc.sync.dma_start(tap_own[:], t1n_own[0:256, :])
            nc.sync.dma_start(tap_t2[:], t2h_own[0:256, :])
            nc.sync.dma_start(tap_ag[:], t2h_all[NSH:NSH + 256, :])
        if PH >= 6:
         with ExitStack() as ctx:
            edge_phase(ctx, "l2", t2h_all[0:SPLIT, :], t2h_all[SPLIT:N, :],
                       t2n_own[:], a2s_sb, evict2)

        # final output
        if PH < 6:
            nc.vector.memset(out_stage[:], 0.0)
        full = (NSH // 128) * 128
        if full:
            nc.sync.dma_start(
                out_d[0:full, :].rearrange("(t p) j -> p t j", p=128),
                out_stage[:, 0:full // 128, :])
        if NSH > full:
            nc.sync.dma_start(out_d[full:NSH, :],
                              out_stage[0:NSH - full, NT - 1, :])

    nc.compile()
    return nc


def _wr_rows(nc, dst, r0, rows, st, width, col0=0):
    """DMA staging [128, G, width] (rows r = g*128+p at [p, g]) to DRAM rows
    dst[r0:r0+rows]. col0: starting tile index inside the staging buffer."""
    g_full = rows // 128
    if g_full:
        nc.sync.dma_start(
            dst[r0:r0 + g_full * 128, :].rearrange("(g p) c -> p g c", p=128),
            st[:, col0:col0 + g_full, :])
    rem = rows - g_full * 128
    if rem:
        nc.sync.dma_start(dst[r0 + g_full * 128:r0 + rows, :],
                          st[0:rem, col0 + g_full, :])


# ----------------------------------------------------------------------------
# entry point
# ----------------------------------------------------------------------------

_CACHE = {}
_RUNNER_CACHE = {}


def _make_runner(nc):
    """Persistent jitted shard_map runner for nc (mirrors
    bass2jax.run_bass_via_pjrt but caches the traced computation so repeat
    kernel() calls skip retrace/recompile; inputs are uploaded per call)."""
    import jax
    import concourse.mybir as mybir_
    from concourse.bass2jax import _bass_exec_p, partition_id_tensor, \
        install_neuronx_cc_hook
    from jax.sharding import Mesh, PartitionSpec, NamedSharding
    from jax.experimental.shard_map import shard_map

    install_neuronx_cc_hook()
    partition_name = (nc.partition_id_tensor.name
                      if nc.partition_id_tensor else None)
    in_names, out_names, out_avals, zero_outs = [], [], [], []
    for alloc in nc.m.functions[0].allocations:
        if not isinstance(alloc, mybir_.MemoryLocationSet):
            continue
        name = alloc.memorylocations[0].name
        if alloc.kind == "ExternalInput":
            if name != partition_name:
                in_names.append(name)
        elif alloc.kind == "ExternalOutput":
            shape = tuple(alloc.tensor_shape)
            dtype = mybir_.dt.np(alloc.dtype)
            out_names.append(name)
            out_avals.append(jax.core.ShapedArray(shape, dtype))
            zero_outs.append(np.zeros(shape, dtype))
    n_params = len(in_names)
    n_outs = len(out_avals)
    all_in = list(in_names) + list(out_names)
    if partition_name is not None:
        all_in.append(partition_name)
    donate = tuple(range(n_params, n_params + n_outs))

    def _body(*args):
        operands = list(args)
        if partition_name is not None:
            operands.append(partition_id_tensor())
        return tuple(_bass_exec_p.bind(
            *operands, out_avals=tuple(out_avals), in_names=tuple(all_in),
            out_names=tuple(out_names), lowering_input_output_aliases=(),
            sim_require_finite=True, sim_require_nnan=True, nc=nc))

    devices = jax.devices()[:NCORES]
    mesh = Mesh(np.asarray(devices), ("core",))
    sharded = jax.jit(
        shard_map(_body, mesh=mesh,
                  in_specs=(PartitionSpec("core"),) * (n_params + n_outs),
                  out_specs=(PartitionSpec("core"),) * n_outs,
                  check_rep=False),
        donate_argnums=donate, keep_unused=True)
    sh = NamedSharding(mesh, PartitionSpec("core"))

    def run(in_maps):
        concat_in = [
            jax.device_put(np.concatenate(
                [np.asarray(in_maps[c][n]) for c in range(NCORES)], axis=0),
                sh)
            for n in in_names]
        zs = [jax.device_put(
            np.zeros((NCORES * z.shape[0], *z.shape[1:]), z.dtype), sh)
            for z in zero_outs]
        outs = sharded(*concat_in, *zs)
        return [{name: np.asarray(outs[i]).reshape(
                    NCORES, *out_avals[i].shape)[c]
                 for i, name in enumerate(out_names)}
                for c in range(NCORES)]

    return run


def kernel(x, edge_index, edge_attr, W1, att_src1, att_dst1, b1,
           W2, att_src2, att_dst2, b2, ff1_w, ff1_b, ff2_w, ff2_b):
    x = np.asarray(x, np.float32)
    edge_index = np.asarray(edge_index)
    args = [np.asarray(a, np.float32) for a in
            (W1, att_src1, att_dst1, b1, W2, att_src2, att_dst2, b2,
             ff1_w, ff1_b, ff2_w, ff2_b)]
    in_maps, sched, dims = _prep(x, edge_index, *args)
    key = (dims["N"], dims["IN"], tuple(sched.n_lo), tuple(sched.n_hi))
    if key not in _CACHE:
        nc_built = _build(sched, dims)
        _CACHE[key] = (nc_built, dims["salt_name"])
    nc, salt_name = _CACHE[key]
    salt = np.zeros((1, 4), np.float32)
    for m in in_maps:
        m[salt_name] = salt
    if key not in _RUNNER_CACHE:
        _RUNNER_CACHE[key] = _make_runner(nc)
    res = _RUNNER_CACHE[key](in_maps)
    out = np.concatenate([res[k]["out"] for k in range(NCORES)], axis=0)
    return out.astype(np.float32)



# revision 28
# speedup vs baseline: 1.0835x; 1.0835x over previous
"""GAT (2-layer GATConv + FF head) on 8 Trainium2 NeuronCores.

Strategy (per sharding hint): nodes + incident edges partitioned by
destination across 8 cores; per-edge softmax/scatter local to the
destination shard via one-hot matmul-scatter into PSUM; small weights
replicated. Layer-1 node features are computed fully replicated (input x
is available everywhere); layer-2 features are computed on the owning
shard and exchanged with a single feature AllGather. Random-access reads
(h[src] rows, own-shard a_dst rows) use SWDGE dma_gather from fp16 DRAM
tables. Per-edge a_src is recomputed on the fly from the gathered rows
(dot with att_src on VectorE), so no global narrow tables or narrow
exchange exist. The layer-2 dense is fused into the layer-1 edge-phase
evict so t2h_own completes with the edge phase and the AllGather fires
immediately; each tile's local nd gather + one-hot build are issued
before its table gathers so local work overlaps the collective.

Message path in fp16 (tables, gathered rows, attention weights, matmul
operands); accumulation in fp32 PSUM; per-edge logits in fp32.
"""
import sys
sys.path.insert(0, "/opt/trn_rl_repo")

import numpy as np
from contextlib import ExitStack

import concourse.bass as bass
import concourse.bacc as bacc
import concourse.tile as tile
import concourse.mybir as mybir
from concourse.bass_utils import run_bass_kernel_spmd

dt = mybir.dt
OP = mybir.AluOpType
ACT = mybir.ActivationFunctionType

NCORES = 8
H = 4
NEG_SLOPE = 0.2


# ----------------------------------------------------------------------------
# host-side prep
# ----------------------------------------------------------------------------

def _wrap_idx(idx):
    """Pack an index list into the SWDGE wrapped layout [128, n/16] int16:
    index i -> partition i%16 (replicated to all 8 16-partition groups),
    free offset i//16."""
    n = len(idx)
    assert n % 128 == 0
    out = np.zeros((128, n // 16), np.int16)
    a = np.asarray(idx, np.int16).reshape(n // 16, 16).T  # [16, n/16]
    for r in range(8):
        out[r * 16:(r + 1) * 16, :] = a
    return out


def _pad128(a, fill):
    n = len(a)
    m = ((n + 127) // 128) * 128
    return np.concatenate([a, np.full(m - n, fill, a.dtype)])


class Sched:
    """Static, core-uniform per-tile chunk schedule."""

    def __init__(self, n_lo, n_hi):
        self.n_lo = n_lo          # [NT] chunks for lo-half gathers
        self.n_hi = n_hi          # [NT] chunks for hi-half gathers
        self.ct = [a + b for a, b in zip(n_lo, n_hi)]
        self.base = np.concatenate([[0], np.cumsum(self.ct)]).astype(int)
        self.total = int(self.base[-1])  # total chunks per core


_EDGE_CACHE = {}


def _prep(x, edge_index, W1, att_src1, att_dst1, b1, W2, att_src2, att_dst2,
          b2, ff1_w, ff1_b, ff2_w, ff2_b):
    N, IN = x.shape
    F = W1.shape[1]               # 256
    C1 = F // H
    C2 = W2.shape[1] // H
    NSH = N // NCORES
    NT = (NSH + 127) // 128
    NLO_T = (N // 2) // 128
    SPLIT = NLO_T * 128

    import hashlib as _hl
    ekey = (N, _hl.sha1(np.ascontiguousarray(edge_index).tobytes())
            .hexdigest())
    cached = _EDGE_CACHE.get(ekey)
    if cached is not None:
        (sched, idx_h, idx_nd, dstloc, idx_own_lo, idx_own_hi,
         own_mask) = cached
        return _prep_finish(x, W1, att_src1, att_dst1, b1, W2, att_src2,
                            att_dst2, b2, ff1_w, ff1_b, ff2_w, ff2_b,
                            sched, idx_h, idx_nd, dstloc, idx_own_lo,
                            idx_own_hi, own_mask,
                            N, IN, F, NSH, NT, SPLIT, C1, C2)

    E = edge_index.shape[1]
    ar = np.arange(N, dtype=np.int64)
    src = np.concatenate([edge_index[0], ar])
    dst = np.concatenate([edge_index[1], ar])

    shard = dst // NSH
    dstloc_all = dst - shard * NSH

    # group edges per (core, tile, half)
    per = [[[None, None] for _ in range(NT)] for _ in range(NCORES)]
    for k in range(NCORES):
        m = shard == k
        s_k, dl_k = src[m], dstloc_all[m]
        t_k = dl_k // 128
        for t in range(NT):
            mt = t_k == t
            s_t, dl_t = s_k[mt], dl_k[mt]
            lo = s_t < SPLIT
            per[k][t][0] = (s_t[lo], dl_t[lo])
            per[k][t][1] = (s_t[~lo] - SPLIT, dl_t[~lo])

    n_lo = [max((len(per[k][t][0][0]) + 127) // 128 for k in range(NCORES))
            for t in range(NT)]
    n_hi = [max((len(per[k][t][1][0]) + 127) // 128 for k in range(NCORES))
            for t in range(NT)]
    sched = Sched(n_lo, n_hi)

    # per-core edge arrays in schedule order
    idx_h = []       # [128, total*8] int16  (gather idx, lo/hi-local rows)
    idx_nd = []      # [128, total*8] int16  (dstlocal shard rows)
    dstloc = []      # [128, total] f32      (tile-local dst or -1)
    for k in range(NCORES):
        ih = np.zeros((128, sched.total * 8), np.int16)
        nd = np.zeros((128, sched.total * 8), np.int16)
        dl = np.full((128, sched.total), -1.0, np.float32)
        for t in range(NT):
            off = sched.base[t]
            for half, nch in ((0, n_lo[t]), (1, n_hi[t])):
                if nch == 0:
                    continue
                s_t, dl_t = per[k][t][half]
                ne = nch * 128
                sp = _pad128(np.concatenate([s_t, np.zeros(ne - len(s_t),
                                                           np.int64)]), 0)[:ne]
                sp[len(s_t):] = 0
                dlp = np.full(ne, -1.0, np.float32)
                dlp[:len(dl_t)] = (dl_t - t * 128).astype(np.float32)
                ndp = np.zeros(ne, np.int64)
                ndp[:len(dl_t)] = dl_t
                ih[:, off * 8:(off + nch) * 8] = _wrap_idx(sp)
                nd[:, off * 8:(off + nch) * 8] = _wrap_idx(ndp)
                dl[:, off:off + nch] = dlp.reshape(nch, 128).T
                off += nch
        idx_h.append(ih)
        idx_nd.append(nd)
        dstloc.append(dl)

    # own-narrow build: gather T1n rows for own shard (lo/hi + select mask)
    own_rows_pad = NT * 128
    idx_own_lo, idx_own_hi, own_mask = [], [], []
    for k in range(NCORES):
        rows = np.arange(k * NSH, (k + 1) * NSH)
        rows = np.concatenate([rows, np.full(own_rows_pad - NSH, rows[0])])
        is_lo = rows < SPLIT
        lo_i = np.where(is_lo, rows, 0)
        hi_i = np.where(is_lo, 0, rows - SPLIT)
        idx_own_lo.append(_wrap_idx(lo_i))
        idx_own_hi.append(_wrap_idx(hi_i))
        m = np.zeros((128, NT), np.float16)
        m[:, :] = is_lo.reshape(NT, 128).T.astype(np.float16)
        own_mask.append(m)

    _EDGE_CACHE[ekey] = (sched, idx_h, idx_nd, dstloc, idx_own_lo,
                         idx_own_hi, own_mask)
    return _prep_finish(x, W1, att_src1, att_dst1, b1, W2, att_src2,
                        att_dst2, b2, ff1_w, ff1_b, ff2_w, ff2_b,
                        sched, idx_h, idx_nd, dstloc, idx_own_lo,
                        idx_own_hi, own_mask,
                        N, IN, F, NSH, NT, SPLIT, C1, C2)


def _prep_finish(x, W1, att_src1, att_dst1, b1, W2, att_src2, att_dst2, b2,
                 ff1_w, ff1_b, ff2_w, ff2_b, sched, idx_h, idx_nd, dstloc,
                 idx_own_lo, idx_own_hi, own_mask,
                 N, IN, F, NSH, NT, SPLIT, C1, C2):
    # weights
    def aug(W, a_s, a_d, C):
        v_s = np.einsum("fhc,hc->fh", W.reshape(-1, H, C), a_s)
        v_d = np.einsum("fhc,hc->fh", W.reshape(-1, H, C), a_d)
        return np.concatenate([W, v_s, v_d], axis=1).astype(np.float16)

    W1aug = aug(W1, att_src1, att_dst1, C1)              # [IN, F+8]
    W2aug = aug(W2, att_src2, att_dst2, C2)              # [F, F+8]
    W2aug_pk = W2aug.reshape(2, 128, F + 8).transpose(1, 0, 2).copy()

    xT16 = np.ascontiguousarray(x.T).astype(np.float16)  # [IN, N]

    iota16 = np.tile(np.arange(128, dtype=np.float16), (128, 1))
    ident16 = np.eye(128, dtype=np.float16)

    const = {
        "xT16": xT16, "W1aug": W1aug, "W2aug": W2aug_pk,
        "iota16": iota16, "ident16": ident16,
        "a1srep": np.tile(att_src1.astype(np.float16).reshape(1, F),
                          (128, 1)),
        "a2srep": np.tile(att_src2.astype(np.float16).reshape(1, F),
                          (128, 1)),
        "b1rep": np.tile(b1.astype(np.float32), (128, 1)),
        "b2rep": np.tile(b2.astype(np.float32), (128, 1)),
        "f1brep": np.tile(ff1_b.astype(np.float32), (128, 1)),
        "f2brep": np.tile(ff2_b.astype(np.float32), (128, 1)),
        "ff1w16": ff1_w.astype(np.float16),
        "ff2w16": ff2_w.astype(np.float16),
    }

    in_maps = []
    for k in range(NCORES):
        m = dict(const)
        m.update({
            "idx_h": idx_h[k], "idx_nd": idx_nd[k], "dstloc": dstloc[k],
            "idx_own_lo": idx_own_lo[k], "idx_own_hi": idx_own_hi[k],
            "own_mask": own_mask[k],
        })
        in_maps.append(m)

    dims = dict(N=N, IN=IN, F=F, NSH=NSH, NT=NT, SPLIT=SPLIT,
                NLO=SPLIT, NHI=N - SPLIT, C2=C2, FH=ff1_w.shape[1])
    return in_maps, sched, dims


# ----------------------------------------------------------------------------
# device program
# ----------------------------------------------------------------------------

def _gather_split(nc, out_ap_fn, tab, idx_sb, n_chunks, elem, q0):
    """Emit dma_gather calls capped at 8 chunks (1024 idxs) each.
    out_ap_fn(c0, c1) -> output AP for chunk range; idx_sb indexed per chunk."""
    c0 = 0
    q = q0
    while c0 < n_chunks:
        c1 = min(c0 + 8, n_chunks)
        nc.gpsimd.dma_gather(
            out_ap_fn(c0, c1), tab, idx_sb[:, c0 * 8:c1 * 8],
            num_idxs=(c1 - c0) * 128, num_idxs_reg=(c1 - c0) * 128,
            elem_size=elem, queue_num=q % 4)
        q += 1
        c0 = c1


def _build(sched, dims):
    import os, hashlib
    PH = int(os.environ.get('K_PHASES', '6'))
    TAPS = int(os.environ.get('K_TAPS', '0'))
    N, IN, F, NSH, NT = dims["N"], dims["IN"], dims["F"], dims["NSH"], dims["NT"]
    NLO, NHI, SPLIT = dims["NLO"], dims["NHI"], dims["SPLIT"]
    C2, FH = dims["C2"], dims["FH"]
    FA = F + 8
    NTG = (N + 127) // 128          # global node tiles
    NTG_LO = NLO // 128

    nc = bacc.Bacc("TRN2", target_bir_lowering=False, num_devices=NCORES,
                   num_swdge_queues=4)
    # The neuronx compile cache keys on the jit signature only (the embedded
    # BIR is not hashed), so two different programs with identical I/O would
    # alias to one NEFF. A content-named dummy input de-aliases them.
    with open(__file__, "rb") as _f:
        _salt = hashlib.sha256(
            _f.read() + repr((sched.n_lo, sched.n_hi, sorted(dims.items()),
                              PH)).encode()).hexdigest()[:16]
    nc.dram_tensor(f"salt_{_salt}", [1, 4], dt.float32, kind="ExternalInput")
    dims["salt_name"] = f"salt_{_salt}"

    # inputs
    xT16 = nc.dram_tensor("xT16", [IN, N], dt.float16, kind="ExternalInput")
    W1aug = nc.dram_tensor("W1aug", [IN, FA], dt.float16, kind="ExternalInput")
    W2aug = nc.dram_tensor("W2aug", [128, 2, FA], dt.float16, kind="ExternalInput")
    iota_in = nc.dram_tensor("iota16", [128, 128], dt.float16, kind="ExternalInput")
    ident_in = nc.dram_tensor("ident16", [128, 128], dt.float16, kind="ExternalInput")
    b1rep = nc.dram_tensor("b1rep", [128, F], dt.float32, kind="ExternalInput")
    b2rep = nc.dram_tensor("b2rep", [128, C2], dt.float32, kind="ExternalInput")
    f1brep = nc.dram_tensor("f1brep", [128, FH], dt.float32, kind="ExternalInput")
    f2brep = nc.dram_tensor("f2brep", [128, 2], dt.float32, kind="ExternalInput")
    ff1w16 = nc.dram_tensor("ff1w16", [C2, FH], dt.float16, kind="ExternalInput")
    ff2w16 = nc.dram_tensor("ff2w16", [FH, 2], dt.float16, kind="ExternalInput")
    a1srep_d = nc.dram_tensor("a1srep", [128, F], dt.float16, kind="ExternalInput")
    a2srep_d = nc.dram_tensor("a2srep", [128, F], dt.float16, kind="ExternalInput")
    idx_h_d = nc.dram_tensor("idx_h", [128, sched.total * 8], dt.int16, kind="ExternalInput")
    idx_nd_d = nc.dram_tensor("idx_nd", [128, sched.total * 8], dt.int16, kind="ExternalInput")
    dstloc_d = nc.dram_tensor("dstloc", [128, sched.total], dt.float32, kind="ExternalInput")
    iol_d = nc.dram_tensor("idx_own_lo", [128, NT * 8], dt.int16, kind="ExternalInput")
    ioh_d = nc.dram_tensor("idx_own_hi", [128, NT * 8], dt.int16, kind="ExternalInput")
    omask_d = nc.dram_tensor("own_mask", [128, NT], dt.float16, kind="ExternalInput")

    out_d = nc.dram_tensor("out", [NSH, 2], dt.float32, kind="ExternalOutput")
    if TAPS:
        tap_h = nc.dram_tensor("tap_h", [256, F], dt.float16, kind="ExternalOutput")
        tap_n = nc.dram_tensor("tap_n", [256, 128], dt.float16, kind="ExternalOutput")
        tap_own = nc.dram_tensor("tap_own", [256, 128], dt.float16, kind="ExternalOutput")
        tap_h1 = nc.dram_tensor("tap_h1", [NSH, F], dt.float16, kind="ExternalOutput")
        tap_t2 = nc.dram_tensor("tap_t2", [256, F], dt.float16, kind="ExternalOutput")
        tap_ag = nc.dram_tensor("tap_ag", [256, F], dt.float16, kind="ExternalOutput")
        tap_h2 = nc.dram_tensor("tap_h2", [NSH, C2], dt.float16, kind="ExternalOutput")
        tap_f1 = nc.dram_tensor("tap_f1", [NSH, FH], dt.float16, kind="ExternalOutput")

    with tile.TileContext(nc) as tc, ExitStack() as octx:
        # persistent pools
        dram = octx.enter_context(tc.tile_pool(name="dram", bufs=1, space="DRAM"))
        cpool = octx.enter_context(tc.tile_pool(name="const", bufs=1))
        stash = octx.enter_context(tc.tile_pool(name="stash", bufs=1))

        # DRAM tables
        t1h_lo = dram.tile([NLO, F], dt.float16)
        t1h_hi = dram.tile([NHI, F], dt.float16)
        t1n_lo = dram.tile([NLO, 128], dt.float16)
        t1n_hi = dram.tile([NHI, 128], dt.float16)
        t1n_own = dram.tile([NT * 128, 128], dt.float16)
        t2h_own = dram.tile([NSH, F], dt.float16)
        t2n_own = dram.tile([NT * 128, 128], dt.float16)
        t2h_all = dram.tile([N, F], dt.float16)
        AGB = [0, 13 * 128, 25 * 128, 37 * 128, NSH]
        ag_sh = [dram.tile([NCORES * (AGB[j + 1] - AGB[j]), F], dt.float16,
                           addr_space="Shared", name=f"ag_sh{j}")
                 for j in range(4)]

        # constants in SBUF
        iota16 = cpool.tile([128, 128], dt.float16)
        nc.sync.dma_start(iota16[:], iota_in[:])
        ident16 = cpool.tile([128, 128], dt.float16)
        nc.sync.dma_start(ident16[:], ident_in[:])
        w1a_sb = cpool.tile([IN, FA], dt.float16)
        nc.sync.dma_start(w1a_sb[:], W1aug[:])
        w2a_sb = cpool.tile([128, 2, FA], dt.float16)
        nc.sync.dma_start(w2a_sb[:], W2aug[:])
        b1_sb = cpool.tile([128, F], dt.float32)
        nc.sync.dma_start(b1_sb[:], b1rep[:])
        b2_sb = cpool.tile([128, C2], dt.float32)
        nc.sync.dma_start(b2_sb[:], b2rep[:])
        f1b_sb = cpool.tile([128, FH], dt.float32)
        nc.sync.dma_start(f1b_sb[:], f1brep[:])
        f2b_sb = cpool.tile([128, 2], dt.float32)
        nc.sync.dma_start(f2b_sb[:], f2brep[:])
        ff1_sb = cpool.tile([C2, FH], dt.float16)
        nc.sync.dma_start(ff1_sb[:], ff1w16[:])
        ff2_sb = cpool.tile([FH, 2], dt.float16)
        nc.sync.dma_start(ff2_sb[:], ff2w16[:])
        a1s_sb = cpool.tile([128, F], dt.float16)
        nc.sync.dma_start(a1s_sb[:], a1srep_d[:])
        a2s_sb = cpool.tile([128, F], dt.float16)
        nc.sync.dma_start(a2s_sb[:], a2srep_d[:])

        # layer-1 hidden transposed, kept in SBUF for the layer-2 dense
        h1T = stash.tile([128, 2, NT, 128], dt.float16)
        out_stage = stash.tile([128, NT, 2], dt.float32)

        # ------------------------------------------------------------------
        # phase A: full replicated layer-1 dense -> T1 tables
        # ------------------------------------------------------------------
        with ExitStack() as ctx:
            xp = ctx.enter_context(tc.tile_pool(name="xp", bufs=2))
            pp = ctx.enter_context(tc.tile_pool(name="pp", bufs=4, space="PSUM"))
            sp = ctx.enter_context(tc.tile_pool(name="sp", bufs=2))

            G = 8
            groups = []
            m0 = 0
            while m0 < NTG:
                g = min(G, NTG - m0)
                groups.append((m0, g))
                m0 += g
            for (m0, g) in groups:
                xs = xp.tile([IN, G * 128], dt.float16, tag="xs")
                rows_t = min(g * 128, N - m0 * 128)
                nc.sync.dma_start(xs[:, 0:rows_t], xT16[:, m0 * 128:m0 * 128 + rows_t])
                hst = sp.tile([128, G, F], dt.float16, tag="hst")
                nst = sp.tile([128, G, 128], dt.float16, tag="nst")
                for j in range(g):
                    m = m0 + j
                    rows = min(128, N - m * 128)
                    ps = pp.tile([128, FA], dt.float32, tag="ps")
                    nc.tensor.matmul(ps[0:rows, :], xs[:, j * 128:j * 128 + rows],
                                     w1a_sb[:], start=True, stop=True)
                    nc.scalar.activation(hst[0:rows, j, :], ps[0:rows, 0:F], ACT.Copy)
                    nc.vector.tensor_copy(nst[0:rows, j, 0:8], ps[0:rows, F:FA])
                # group DMA to tables (handle a group straddling SPLIT)
                r0 = m0 * 128
                rows_t = min(g * 128, N - r0)
                if r0 + rows_t <= SPLIT:
                    dst_h, dst_n, rr = t1h_lo, t1n_lo, r0
                elif r0 >= SPLIT:
                    dst_h, dst_n, rr = t1h_hi, t1n_hi, r0 - SPLIT
                else:
                    dst_h = None
                if dst_h is not None:
                    _wr_rows(nc, dst_h, rr, rows_t, hst, F)
                    _wr_rows(nc, dst_n, rr, rows_t, nst, 128)
                else:
                    a = SPLIT - r0
                    _wr_rows(nc, t1h_lo, r0, a, hst, F)
                    _wr_rows(nc, t1n_lo, r0, a, nst, 128)
                    _wr_rows(nc, t1h_hi, 0, rows_t - a, hst, F, col0=a // 128)
                    _wr_rows(nc, t1n_hi, 0, rows_t - a, nst, 128, col0=a // 128)

        # ------------------------------------------------------------------
        # own-narrow table for layer 1 (gather own rows from t1n lo/hi)
        # ------------------------------------------------------------------
        if PH >= 2:
         with ExitStack() as ctx:
            op_ = ctx.enter_context(tc.tile_pool(name="op", bufs=1))
            il = op_.tile([128, NT * 8], dt.int16)
            nc.sync.dma_start(il[:], iol_d[:])
            ih = op_.tile([128, NT * 8], dt.int16)
            nc.sync.dma_start(ih[:], ioh_d[:])
            msk = op_.tile([128, NT], dt.float16)
            nc.sync.dma_start(msk[:], omask_d[:])
            glo = op_.tile([128, NT, 128], dt.float16)
            _gather_split(nc, lambda a, b: glo[:, a:b, :], t1n_lo[:], il, NT,
                          128, 0)
            ghi = op_.tile([128, NT, 128], dt.float16)
            _gather_split(nc, lambda a, b: ghi[:, a:b, :], t1n_hi[:], ih, NT,
                          128, 1)
            mrg = op_.tile([128, NT, 128], dt.float16)
            # mrg = ghi + (glo - ghi) * mask
            nc.vector.tensor_tensor(mrg[:], glo[:], ghi[:], op=OP.subtract)
            nc.vector.tensor_tensor(
                mrg[:], mrg[:],
                msk[:].unsqueeze(2).broadcast_to([128, NT, 128]), op=OP.mult)
            nc.vector.tensor_tensor(mrg[:], mrg[:], ghi[:], op=OP.add)
            nc.sync.dma_start(
                t1n_own[:].rearrange("(t p) c -> p t c", p=128), mrg[:])

        # ------------------------------------------------------------------
        # edge phases
        # ------------------------------------------------------------------
        def edge_phase(ctx, name, tab_lo, tab_hi, n_own, asrep_sb, evict,
                       post_tile=None):
            ep = ctx.enter_context(tc.tile_pool(name=name + "e", bufs=2))
            pp = ctx.enter_context(tc.tile_pool(name=name + "p", bufs=2, space="PSUM"))
            for t in range(NT):
                ct = sched.ct[t]
                if ct == 0:
                    if post_tile is not None:
                        post_tile(t)
                    continue
                nlo, nhi = sched.n_lo[t], sched.n_hi[t]
                b0 = sched.base[t]
                ixh = ep.tile([128, ct * 8], dt.int16, tag="ixh")
                nc.sync.dma_start(ixh[:], idx_h_d[:, b0 * 8:(b0 + ct) * 8])
                ixn = ep.tile([128, ct * 8], dt.int16, tag="ixn")
                nc.sync.dma_start(ixn[:], idx_nd_d[:, b0 * 8:(b0 + ct) * 8])
                dl = ep.tile([128, ct], dt.float32, tag="dl")
                nc.sync.dma_start(dl[:], dstloc_d[:, b0:b0 + ct])

                # local-table gather + one-hot build first: at the layer-2
                # boundary these depend only on phase-C outputs, so they can
                # proceed while the AllGather (which gates the g gathers
                # below) is still in flight.
                g = ep.tile([128, ct, F], dt.float16, tag="g")
                nd_ = ep.tile([128, ct, 128], dt.float16, tag="nd")
                _gather_split(nc, lambda a, b: nd_[:, a:b, :], n_own, ixn,
                              ct, 128, 1)
                oh = ep.tile([128, ct, 128], dt.float16, tag="oh")
                nc.vector.tensor_tensor(
                    oh[:],
                    iota16[:].unsqueeze(1).broadcast_to([128, ct, 128]),
                    dl[:].unsqueeze(2).broadcast_to([128, ct, 128]),
                    op=OP.is_equal)
                if nlo:
                    _gather_split(nc, lambda a, b: g[:, a:b, :], tab_lo,
                                  ixh, nlo, F, 0)
                if nhi:
                    _gather_split(
                        nc, lambda a, b: g[:, nlo + a:nlo + b, :], tab_hi,
                        ixh[:, nlo * 8:ct * 8], nhi, F, 2)

                # a_src per edge from the gathered rows: ns[e,h] =
                # sum_c g[e,h*C+c] * att_src[h,c]; rhs[:, :, 0:F] is scratch
                # for the product (overwritten later by the fold).
                rhs = ep.tile([128, ct, F + H], dt.float16, tag="rhs")
                nc.vector.tensor_tensor(
                    rhs[:, :, 0:F], g[:],
                    asrep_sb[:].unsqueeze(1).broadcast_to([128, ct, F]),
                    op=OP.mult)
                nsr = ep.tile([128, ct, H], dt.float32, tag="nsr")
                nc.vector.tensor_reduce(
                    nsr[:], rhs[:, :, 0:F].rearrange("p c (h d) -> p c h d",
                                                     h=H),
                    axis=mybir.AxisListType.X, op=OP.add)

                # narrow: alpha = lrelu(a_src + a_dst); ex = exp(alpha)
                alpha = ep.tile([128, ct, H], dt.float32, tag="alpha")
                nc.vector.tensor_tensor(alpha[:], nsr[:], nd_[:, :, H:2 * H],
                                        op=OP.add)
                nc.vector.scalar_tensor_tensor(
                    alpha[:], alpha[:], float(NEG_SLOPE), alpha[:],
                    op0=OP.mult, op1=OP.max)
                nc.scalar.activation(rhs[:, :, F:F + H], alpha[:], ACT.Exp)
                # fold: rhs[:, :, 0:F] = g * ex (per-head broadcast)
                nc.vector.tensor_tensor(
                    rhs[:, :, 0:F].rearrange("p c (h d) -> p c h d", h=H),
                    g[:].rearrange("p c (h d) -> p c h d", h=H),
                    rhs[:, :, F:F + H].unsqueeze(3).broadcast_to(
                        [128, ct, H, F // H]),
                    op=OP.mult)
                # matmul-scatter (one-hot built above, before the g gathers)
                ps = pp.tile([128, F + H], dt.float32, tag="ps")
                for c in range(ct):
                    nc.tensor.matmul(ps[:], oh[:, c, :], rhs[:, c, :],
                                     start=(c == 0), stop=(c == ct - 1))
                evict(ep, pp, t, ps)
                if post_tile is not None:
                    post_tile(t)

        # ---- layer 1 evict: h1 = relu(agg/den + b1); build h1T + h1own ----
        def evict1(ep, pp, t, ps):
            rows = min(128, NSH - t * 128)
            rcp = ep.tile([128, H], dt.float32, tag="rcp")
            nc.vector.reciprocal(rcp[:], ps[:, F:F + H])
            pre = ep.tile([128, F], dt.float32, tag="pre")
            nc.vector.tensor_tensor(
                pre[:].rearrange("p (h d) -> p h d", h=H),
                ps[:, 0:F].rearrange("p (h d) -> p h d", h=H),
                rcp[:].unsqueeze(2).broadcast_to([128, H, F // H]), op=OP.mult)
            nc.vector.tensor_tensor(pre[:], pre[:], b1_sb[:], op=OP.add)
            h1r = ep.tile([128, F], dt.float16, tag="h1r")
            nc.scalar.activation(h1r[:], pre[:], ACT.Relu)
            if TAPS:
                nc.sync.dma_start(tap_h1[t * 128:t * 128 + rows, :], h1r[0:rows, :])
            for b in range(2):
                tp = pp.tile([128, 128], dt.float16, tag="tp")
                nc.tensor.transpose(tp[:], h1r[:, b * 128:(b + 1) * 128], ident16[:])
                nc.scalar.activation(h1T[:, b, t, :], tp[:], ACT.Copy)
            # fused layer-2 dense for this tile: t2h_own rows are complete as
            # soon as the L1 edge phase finishes, so the AllGather can launch
            # without a separate dense pass in between.
            ps2 = pp.tile([128, FA], dt.float32, tag="ps2")
            for b in range(2):
                nc.tensor.matmul(ps2[:], h1T[:, b, t, :], w2a_sb[:, b, :],
                                 start=(b == 0), stop=(b == 1))
            hst = ep.tile([128, F], dt.float16, tag="hst")
            nc.scalar.activation(hst[:], ps2[:, 0:F], ACT.Copy)
            nst = ep.tile([128, 128], dt.float16, tag="nst")
            nc.vector.tensor_copy(nst[:, 0:8], ps2[:, F:FA])
            nc.sync.dma_start(t2h_own[t * 128:t * 128 + rows, :],
                              hst[0:rows, :])
            nc.sync.dma_start(t2n_own[t * 128:(t + 1) * 128, :], nst[:])

        # Chunked inter-layer AllGather: t2h_own rows complete per tile (the
        # layer-2 dense is fused into evict1), so gather them in 4 row
        # chunks fired from inside the L1 edge phase — chunks 1-3 move on
        # the wire while later tiles are still being aggregated. Each chunk
        # has its own Shared output (the framework allows one writer per
        # Shared tensor); rank-major chunks are re-laid node-major into
        # t2h_all by plain contiguous copies after the loop.
        def l1_post_tile(t):
            if PH < 5:
                return
            hi = min((t + 1) * 128, NSH)
            for j in range(4):
                if AGB[j + 1] == hi:
                    nc.gpsimd.collective_compute(
                        "AllGather", OP.bypass,
                        replica_groups=[list(range(NCORES))],
                        ins=[t2h_own[AGB[j]:AGB[j + 1], :].opt()],
                        outs=[ag_sh[j][:].opt()])

        if PH >= 3:
         with ExitStack() as ctx:
            edge_phase(ctx, "l1", t1h_lo[:], t1h_hi[:], t1n_own[:], a1s_sb,
                       evict1, post_tile=l1_post_tile)

        # ------------------------------------------------------------------
        # (layer-2 dense is fused into evict1 above)
        # Re-lay the rank-major AG chunks node-major (chunk j of rank k ->
        # rows k*NSH+AGB[j]). Chunks 1-3 finished on the wire during the L1
        # edge phase, so these copies start immediately.
        for j in (range(4) if PH >= 5 else []):
            rows_j = AGB[j + 1] - AGB[j]
            for k in range(NCORES):
                nc.sync.dma_start(
                    t2h_all[k * NSH + AGB[j]:k * NSH + AGB[j + 1], :],
                    ag_sh[j][k * rows_j:(k + 1) * rows_j, :])

        # ---- layer 2 evict: h2 = relu(mean_h(agg/den) + b2); FF head ----
        def evict2(ep, pp, t, ps):
            rows = min(128, NSH - t * 128)
            rcp = ep.tile([128, H], dt.float32, tag="rcp")
            nc.vector.reciprocal(rcp[:], ps[:, F:F + H])
            pre = ep.tile([128, H, C2], dt.float32, tag="pre")
            nc.vector.tensor_tensor(
                pre[:], ps[:, 0:F].rearrange("p (h d) -> p h d", h=H),
                rcp[:].unsqueeze(2).broadcast_to([128, H, C2]), op=OP.mult)
            red = ep.tile([128, C2], dt.float32, tag="red")
            nc.vector.tensor_reduce(red[:], pre[:].transpose([0, 2, 1]),
                                    axis=mybir.AxisListType.X, op=OP.add)
            nc.vector.scalar_tensor_tensor(red[:], red[:], 1.0 / H, b2_sb[:],
                                           op0=OP.mult, op1=OP.add)
            h2 = ep.tile([128, 128], dt.float16, tag="h2")
            nc.vector.memset(h2[:, C2:128], 0.0)
            nc.scalar.activation(h2[:, 0:C2], red[:], ACT.Relu)
            if TAPS:
                nc.sync.dma_start(tap_h2[t * 128:t * 128 + rows, :],
                                  h2[0:rows, 0:C2])
            # FF: out = relu(h2 @ ff1 + b1f) @ ff2 + b2f  (square transposes)
            tp = pp.tile([128, 128], dt.float16, tag="tp2", bufs=1)
            nc.tensor.transpose(tp[:], h2[:], ident16[:])
            h2T = ep.tile([C2, 128], dt.float16, tag="h2T")
            nc.scalar.activation(h2T[:], tp[0:C2, :], ACT.Copy)
            pf1 = pp.tile([128, FH], dt.float32, tag="pf1", bufs=1)
            nc.tensor.matmul(pf1[:], h2T[:], ff1_sb[:], start=True, stop=True)
            f1p = ep.tile([128, FH], dt.float32, tag="f1p")
            nc.vector.tensor_tensor(f1p[:], pf1[:], f1b_sb[:], op=OP.add)
            f1 = ep.tile([128, 128], dt.float16, tag="f1")
            nc.vector.memset(f1[:, FH:128], 0.0)
            nc.scalar.activation(f1[:, 0:FH], f1p[:], ACT.Relu)
            if TAPS:
                nc.sync.dma_start(tap_f1[t * 128:t * 128 + rows, :],
                                  f1[0:rows, 0:FH])
            tpf = pp.tile([128, 128], dt.float16, tag="tpf", bufs=1)
            nc.tensor.transpose(tpf[:], f1[:], ident16[:])
            f1T = ep.tile([FH, 128], dt.float16, tag="f1T")
            nc.scalar.activation(f1T[:], tpf[0:FH, :], ACT.Copy)
            pf2 = pp.tile([128, 2], dt.float32, tag="pf2", bufs=1)
            nc.tensor.matmul(pf2[:], f1T[:], ff2_sb[:], start=True, stop=True)
            nc.vector.tensor_tensor(out_stage[:, t, :], pf2[:], f2b_sb[:],
                                    op=OP.add)

        if TAPS:
            A = SPLIT - 128
            nc.sync.dma_start(tap_h[0:128, :], t1h_lo[A:A + 128, :])
            nc.sync.dma_start(tap_h[128:256, :], t1h_hi[0:128, :])
            nc.sync.dma_start(tap_n[0:128, :], t1n_lo[A:A + 128, :])
            nc.sync.dma_start(tap_n[128:256, :], t1n_hi[0:128, :])
            nc.sync.dma_start(tap_own[:], t1n_own[0:256, :])
            nc.sync.dma_start(tap_t2[:], t2h_own[0:256, :])
            nc.sync.dma_start(tap_ag[:], t2h_all[NSH:NSH + 256, :])
        if PH >= 6:
         with ExitStack() as ctx:
            edge_phase(ctx, "l2", t2h_all[0:SPLIT, :], t2h_all[SPLIT:N, :],
                       t2n_own[:], a2s_sb, evict2)

        # final output
        if PH < 6:
            nc.vector.memset(out_stage[:], 0.0)
        full = (NSH // 128) * 128
        if full:
            nc.sync.dma_start(
                out_d[0:full, :].rearrange("(t p) j -> p t j", p=128),
                out_stage[:, 0:full // 128, :])
        if NSH > full:
            nc.sync.dma_start(out_d[full:NSH, :],
                              out_stage[0:NSH - full, NT - 1, :])

    nc.compile()
    return nc


def _wr_rows(nc, dst, r0, rows, st, width, col0=0):
    """DMA staging [128, G, width] (rows r = g*128+p at [p, g]) to DRAM rows
    dst[r0:r0+rows]. col0: starting tile index inside the staging buffer."""
    g_full = rows // 128
    if g_full:
        nc.sync.dma_start(
            dst[r0:r0 + g_full * 128, :].rearrange("(g p) c -> p g c", p=128),
            st[:, col0:col0 + g_full, :])
    rem = rows - g_full * 128
    if rem:
        nc.sync.dma_start(dst[r0 + g_full * 128:r0 + rows, :],
                          st[0:rem, col0 + g_full, :])


# ----------------------------------------------------------------------------
# entry point
# ----------------------------------------------------------------------------

_CACHE = {}
_RUNNER_CACHE = {}


def _make_runner(nc):
    """Persistent jitted shard_map runner for nc (mirrors
    bass2jax.run_bass_via_pjrt but caches the traced computation so repeat
    kernel() calls skip retrace/recompile; inputs are uploaded per call)."""
    import jax
    import concourse.mybir as mybir_
    from concourse.bass2jax import _bass_exec_p, partition_id_tensor, \
        install_neuronx_cc_hook
    from jax.sharding import Mesh, PartitionSpec, NamedSharding
    from jax.experimental.shard_map import shard_map

    install_neuronx_cc_hook()
    partition_name = (nc.partition_id_tensor.name
                      if nc.partition_id_tensor else None)
    in_names, out_names, out_avals, zero_outs = [], [], [], []
    for alloc in nc.m.functions[0].allocations:
        if not isinstance(alloc, mybir_.MemoryLocationSet):
            continue
        name = alloc.memorylocations[0].name
        if alloc.kind == "ExternalInput":
            if name != partition_name:
                in_names.append(name)
        elif alloc.kind == "ExternalOutput":
            shape = tuple(alloc.tensor_shape)
            dtype = mybir_.dt.np(alloc.dtype)
            out_names.append(name)
            out_avals.append(jax.core.ShapedArray(shape, dtype))
            zero_outs.append(np.zeros(shape, dtype))
    n_params = len(in_names)
    n_outs = len(out_avals)
    all_in = list(in_names) + list(out_names)
    if partition_name is not None:
        all_in.append(partition_name)
    donate = tuple(range(n_params, n_params + n_outs))

    def _body(*args):
        operands = list(args)
        if partition_name is not None:
            operands.append(partition_id_tensor())
        return tuple(_bass_exec_p.bind(
            *operands, out_avals=tuple(out_avals), in_names=tuple(all_in),
            out_names=tuple(out_names), lowering_input_output_aliases=(),
            sim_require_finite=True, sim_require_nnan=True, nc=nc))

    devices = jax.devices()[:NCORES]
    mesh = Mesh(np.asarray(devices), ("core",))
    sharded = jax.jit(
        shard_map(_body, mesh=mesh,
                  in_specs=(PartitionSpec("core"),) * (n_params + n_outs),
                  out_specs=(PartitionSpec("core"),) * n_outs,
                  check_rep=False),
        donate_argnums=donate, keep_unused=True)
    sh = NamedSharding(mesh, PartitionSpec("core"))

    def run(in_maps):
        concat_in = [
            jax.device_put(np.concatenate(
                [np.asarray(in_maps[c][n]) for c in range(NCORES)], axis=0),
                sh)
            for n in in_names]
        zs = [jax.device_put(
            np.zeros((NCORES * z.shape[0], *z.shape[1:]), z.dtype), sh)
            for z in zero_outs]
        outs = sharded(*concat_in, *zs)
        return [{name: np.asarray(outs[i]).reshape(
                    NCORES, *out_avals[i].shape)[c]
                 for i, name in enumerate(out_names)}
                for c in range(NCORES)]

    return run


def kernel(x, edge_index, edge_attr, W1, att_src1, att_dst1, b1,
           W2, att_src2, att_dst2, b2, ff1_w, ff1_b, ff2_w, ff2_b):
    x = np.asarray(x, np.float32)
    edge_index = np.asarray(edge_index)
    args = [np.asarray(a, np.float32) for a in
            (W1, att_src1, att_dst1, b1, W2, att_src2, att_dst2, b2,
             ff1_w, ff1_b, ff2_w, ff2_b)]
    in_maps, sched, dims = _prep(x, edge_index, *args)
    key = (dims["N"], dims["IN"], tuple(sched.n_lo), tuple(sched.n_hi))
    if key not in _CACHE:
        nc_built = _build(sched, dims)
        _CACHE[key] = (nc_built, dims["salt_name"])
    nc, salt_name = _CACHE[key]
    salt = np.zeros((1, 4), np.float32)
    for m in in_maps:
        m[salt_name] = salt
    if key not in _RUNNER_CACHE:
        _RUNNER_CACHE[key] = _make_runner(nc)
    res = _RUNNER_CACHE[key](in_maps)
    out = np.concatenate([res[k]["out"] for k in range(NCORES)], axis=0)
    return out.astype(np.float32)



# revision 29
# speedup vs baseline: 1.0843x; 1.0007x over previous
"""GAT (2-layer GATConv + FF head) on 8 Trainium2 NeuronCores.

Strategy (per sharding hint): nodes + incident edges partitioned by
destination across 8 cores; per-edge softmax/scatter local to the
destination shard via one-hot matmul-scatter into PSUM; small weights
replicated. Layer-1 node features are computed fully replicated (input x
is available everywhere); layer-2 features are computed on the owning
shard and exchanged with a single feature AllGather. Random-access reads
(h[src] rows, own-shard a_dst rows) use SWDGE dma_gather from fp16 DRAM
tables. Per-edge a_src is recomputed on the fly from the gathered rows
(dot with att_src on VectorE), so no global narrow tables or narrow
exchange exist. The layer-2 dense is fused into the layer-1 edge-phase
evict so t2h_own completes with the edge phase and the AllGather fires
immediately; each tile's local nd gather + one-hot build are issued
before its table gathers so local work overlaps the collective.

Message path in fp16 (tables, gathered rows, attention weights, matmul
operands); accumulation in fp32 PSUM; per-edge logits in fp32.
"""
import sys
sys.path.insert(0, "/opt/trn_rl_repo")

import numpy as np
from contextlib import ExitStack

import concourse.bass as bass
import concourse.bacc as bacc
import concourse.tile as tile
import concourse.mybir as mybir
from concourse.bass_utils import run_bass_kernel_spmd

dt = mybir.dt
OP = mybir.AluOpType
ACT = mybir.ActivationFunctionType

NCORES = 8
H = 4
NEG_SLOPE = 0.2


# ----------------------------------------------------------------------------
# host-side prep
# ----------------------------------------------------------------------------

def _wrap_idx(idx):
    """Pack an index list into the SWDGE wrapped layout [128, n/16] int16:
    index i -> partition i%16 (replicated to all 8 16-partition groups),
    free offset i//16."""
    n = len(idx)
    assert n % 128 == 0
    out = np.zeros((128, n // 16), np.int16)
    a = np.asarray(idx, np.int16).reshape(n // 16, 16).T  # [16, n/16]
    for r in range(8):
        out[r * 16:(r + 1) * 16, :] = a
    return out


def _pad128(a, fill):
    n = len(a)
    m = ((n + 127) // 128) * 128
    return np.concatenate([a, np.full(m - n, fill, a.dtype)])


class Sched:
    """Static, core-uniform per-tile chunk schedule."""

    def __init__(self, n_lo, n_hi):
        self.n_lo = n_lo          # [NT] chunks for lo-half gathers
        self.n_hi = n_hi          # [NT] chunks for hi-half gathers
        self.ct = [a + b for a, b in zip(n_lo, n_hi)]
        self.base = np.concatenate([[0], np.cumsum(self.ct)]).astype(int)
        self.total = int(self.base[-1])  # total chunks per core


_EDGE_CACHE = {}


def _prep(x, edge_index, W1, att_src1, att_dst1, b1, W2, att_src2, att_dst2,
          b2, ff1_w, ff1_b, ff2_w, ff2_b):
    N, IN = x.shape
    F = W1.shape[1]               # 256
    C1 = F // H
    C2 = W2.shape[1] // H
    NSH = N // NCORES
    NT = (NSH + 127) // 128
    NLO_T = (N // 2) // 128
    SPLIT = NLO_T * 128

    import hashlib as _hl
    ekey = (N, _hl.sha1(np.ascontiguousarray(edge_index).tobytes())
            .hexdigest())
    cached = _EDGE_CACHE.get(ekey)
    if cached is not None:
        (sched, idx_h, idx_nd, dstloc, idx_own_lo, idx_own_hi,
         own_mask) = cached
        return _prep_finish(x, W1, att_src1, att_dst1, b1, W2, att_src2,
                            att_dst2, b2, ff1_w, ff1_b, ff2_w, ff2_b,
                            sched, idx_h, idx_nd, dstloc, idx_own_lo,
                            idx_own_hi, own_mask,
                            N, IN, F, NSH, NT, SPLIT, C1, C2)

    E = edge_index.shape[1]
    ar = np.arange(N, dtype=np.int64)
    src = np.concatenate([edge_index[0], ar])
    dst = np.concatenate([edge_index[1], ar])

    shard = dst // NSH
    dstloc_all = dst - shard * NSH

    # group edges per (core, tile, half)
    per = [[[None, None] for _ in range(NT)] for _ in range(NCORES)]
    for k in range(NCORES):
        m = shard == k
        s_k, dl_k = src[m], dstloc_all[m]
        t_k = dl_k // 128
        for t in range(NT):
            mt = t_k == t
            s_t, dl_t = s_k[mt], dl_k[mt]
            lo = s_t < SPLIT
            per[k][t][0] = (s_t[lo], dl_t[lo])
            per[k][t][1] = (s_t[~lo] - SPLIT, dl_t[~lo])

    n_lo = [max((len(per[k][t][0][0]) + 127) // 128 for k in range(NCORES))
            for t in range(NT)]
    n_hi = [max((len(per[k][t][1][0]) + 127) // 128 for k in range(NCORES))
            for t in range(NT)]
    sched = Sched(n_lo, n_hi)

    # per-core edge arrays in schedule order
    idx_h = []       # [128, total*8] int16  (gather idx, lo/hi-local rows)
    idx_nd = []      # [128, total*8] int16  (dstlocal shard rows)
    dstloc = []      # [128, total] f32      (tile-local dst or -1)
    for k in range(NCORES):
        ih = np.zeros((128, sched.total * 8), np.int16)
        nd = np.zeros((128, sched.total * 8), np.int16)
        dl = np.full((128, sched.total), -1.0, np.float32)
        for t in range(NT):
            off = sched.base[t]
            for half, nch in ((0, n_lo[t]), (1, n_hi[t])):
                if nch == 0:
                    continue
                s_t, dl_t = per[k][t][half]
                ne = nch * 128
                sp = _pad128(np.concatenate([s_t, np.zeros(ne - len(s_t),
                                                           np.int64)]), 0)[:ne]
                sp[len(s_t):] = 0
                dlp = np.full(ne, -1.0, np.float32)
                dlp[:len(dl_t)] = (dl_t - t * 128).astype(np.float32)
                ndp = np.zeros(ne, np.int64)
                ndp[:len(dl_t)] = dl_t
                ih[:, off * 8:(off + nch) * 8] = _wrap_idx(sp)
                nd[:, off * 8:(off + nch) * 8] = _wrap_idx(ndp)
                dl[:, off:off + nch] = dlp.reshape(nch, 128).T
                off += nch
        idx_h.append(ih)
        idx_nd.append(nd)
        dstloc.append(dl)

    # own-narrow build: gather T1n rows for own shard (lo/hi + select mask)
    own_rows_pad = NT * 128
    idx_own_lo, idx_own_hi, own_mask = [], [], []
    for k in range(NCORES):
        rows = np.arange(k * NSH, (k + 1) * NSH)
        rows = np.concatenate([rows, np.full(own_rows_pad - NSH, rows[0])])
        is_lo = rows < SPLIT
        lo_i = np.where(is_lo, rows, 0)
        hi_i = np.where(is_lo, 0, rows - SPLIT)
        idx_own_lo.append(_wrap_idx(lo_i))
        idx_own_hi.append(_wrap_idx(hi_i))
        m = np.zeros((128, NT), np.float16)
        m[:, :] = is_lo.reshape(NT, 128).T.astype(np.float16)
        own_mask.append(m)

    _EDGE_CACHE[ekey] = (sched, idx_h, idx_nd, dstloc, idx_own_lo,
                         idx_own_hi, own_mask)
    return _prep_finish(x, W1, att_src1, att_dst1, b1, W2, att_src2,
                        att_dst2, b2, ff1_w, ff1_b, ff2_w, ff2_b,
                        sched, idx_h, idx_nd, dstloc, idx_own_lo,
                        idx_own_hi, own_mask,
                        N, IN, F, NSH, NT, SPLIT, C1, C2)


def _prep_finish(x, W1, att_src1, att_dst1, b1, W2, att_src2, att_dst2, b2,
                 ff1_w, ff1_b, ff2_w, ff2_b, sched, idx_h, idx_nd, dstloc,
                 idx_own_lo, idx_own_hi, own_mask,
                 N, IN, F, NSH, NT, SPLIT, C1, C2):
    # weights
    def aug(W, a_s, a_d, C):
        v_s = np.einsum("fhc,hc->fh", W.reshape(-1, H, C), a_s)
        v_d = np.einsum("fhc,hc->fh", W.reshape(-1, H, C), a_d)
        return np.concatenate([W, v_s, v_d], axis=1).astype(np.float16)

    W1aug = aug(W1, att_src1, att_dst1, C1)              # [IN, F+8]
    W2aug = aug(W2, att_src2, att_dst2, C2)              # [F, F+8]
    W2aug_pk = W2aug.reshape(2, 128, F + 8).transpose(1, 0, 2).copy()

    xT16 = np.ascontiguousarray(x.T).astype(np.float16)  # [IN, N]

    iota16 = np.tile(np.arange(128, dtype=np.float16), (128, 1))
    ident16 = np.eye(128, dtype=np.float16)

    const = {
        "xT16": xT16, "W1aug": W1aug, "W2aug": W2aug_pk,
        "iota16": iota16, "ident16": ident16,
        "a1srep": np.tile(att_src1.astype(np.float16).reshape(1, F),
                          (128, 1)),
        "a2srep": np.tile(att_src2.astype(np.float16).reshape(1, F),
                          (128, 1)),
        "b1rep": np.tile(b1.astype(np.float32), (128, 1)),
        "b2rep": np.tile(b2.astype(np.float32), (128, 1)),
        "f1brep": np.tile(ff1_b.astype(np.float32), (128, 1)),
        "f2brep": np.tile(ff2_b.astype(np.float32), (128, 1)),
        "ff1w16": ff1_w.astype(np.float16),
        "ff2w16": ff2_w.astype(np.float16),
    }

    in_maps = []
    for k in range(NCORES):
        m = dict(const)
        m.update({
            "idx_h": idx_h[k], "idx_nd": idx_nd[k], "dstloc": dstloc[k],
            "idx_own_lo": idx_own_lo[k], "idx_own_hi": idx_own_hi[k],
            "own_mask": own_mask[k],
        })
        in_maps.append(m)

    dims = dict(N=N, IN=IN, F=F, NSH=NSH, NT=NT, SPLIT=SPLIT,
                NLO=SPLIT, NHI=N - SPLIT, C2=C2, FH=ff1_w.shape[1])
    return in_maps, sched, dims


# ----------------------------------------------------------------------------
# device program
# ----------------------------------------------------------------------------

def _gather_split(nc, out_ap_fn, tab, idx_sb, n_chunks, elem, q0):
    """Emit dma_gather calls capped at 8 chunks (1024 idxs) each.
    out_ap_fn(c0, c1) -> output AP for chunk range; idx_sb indexed per chunk."""
    c0 = 0
    q = q0
    while c0 < n_chunks:
        c1 = min(c0 + 8, n_chunks)
        nc.gpsimd.dma_gather(
            out_ap_fn(c0, c1), tab, idx_sb[:, c0 * 8:c1 * 8],
            num_idxs=(c1 - c0) * 128, num_idxs_reg=(c1 - c0) * 128,
            elem_size=elem, queue_num=q % 4)
        q += 1
        c0 = c1


def _build(sched, dims):
    import os, hashlib
    PH = int(os.environ.get('K_PHASES', '6'))
    TAPS = int(os.environ.get('K_TAPS', '0'))
    N, IN, F, NSH, NT = dims["N"], dims["IN"], dims["F"], dims["NSH"], dims["NT"]
    NLO, NHI, SPLIT = dims["NLO"], dims["NHI"], dims["SPLIT"]
    C2, FH = dims["C2"], dims["FH"]
    FA = F + 8
    NTG = (N + 127) // 128          # global node tiles
    NTG_LO = NLO // 128

    nc = bacc.Bacc("TRN2", target_bir_lowering=False, num_devices=NCORES,
                   num_swdge_queues=4)
    # The neuronx compile cache keys on the jit signature only (the embedded
    # BIR is not hashed), so two different programs with identical I/O would
    # alias to one NEFF. A content-named dummy input de-aliases them.
    with open(__file__, "rb") as _f:
        _salt = hashlib.sha256(
            _f.read() + repr((sched.n_lo, sched.n_hi, sorted(dims.items()),
                              PH)).encode()).hexdigest()[:16]
    nc.dram_tensor(f"salt_{_salt}", [1, 4], dt.float32, kind="ExternalInput")
    dims["salt_name"] = f"salt_{_salt}"

    # inputs
    xT16 = nc.dram_tensor("xT16", [IN, N], dt.float16, kind="ExternalInput")
    W1aug = nc.dram_tensor("W1aug", [IN, FA], dt.float16, kind="ExternalInput")
    W2aug = nc.dram_tensor("W2aug", [128, 2, FA], dt.float16, kind="ExternalInput")
    iota_in = nc.dram_tensor("iota16", [128, 128], dt.float16, kind="ExternalInput")
    ident_in = nc.dram_tensor("ident16", [128, 128], dt.float16, kind="ExternalInput")
    b1rep = nc.dram_tensor("b1rep", [128, F], dt.float32, kind="ExternalInput")
    b2rep = nc.dram_tensor("b2rep", [128, C2], dt.float32, kind="ExternalInput")
    f1brep = nc.dram_tensor("f1brep", [128, FH], dt.float32, kind="ExternalInput")
    f2brep = nc.dram_tensor("f2brep", [128, 2], dt.float32, kind="ExternalInput")
    ff1w16 = nc.dram_tensor("ff1w16", [C2, FH], dt.float16, kind="ExternalInput")
    ff2w16 = nc.dram_tensor("ff2w16", [FH, 2], dt.float16, kind="ExternalInput")
    a1srep_d = nc.dram_tensor("a1srep", [128, F], dt.float16, kind="ExternalInput")
    a2srep_d = nc.dram_tensor("a2srep", [128, F], dt.float16, kind="ExternalInput")
    idx_h_d = nc.dram_tensor("idx_h", [128, sched.total * 8], dt.int16, kind="ExternalInput")
    idx_nd_d = nc.dram_tensor("idx_nd", [128, sched.total * 8], dt.int16, kind="ExternalInput")
    dstloc_d = nc.dram_tensor("dstloc", [128, sched.total], dt.float32, kind="ExternalInput")
    iol_d = nc.dram_tensor("idx_own_lo", [128, NT * 8], dt.int16, kind="ExternalInput")
    ioh_d = nc.dram_tensor("idx_own_hi", [128, NT * 8], dt.int16, kind="ExternalInput")
    omask_d = nc.dram_tensor("own_mask", [128, NT], dt.float16, kind="ExternalInput")

    out_d = nc.dram_tensor("out", [NSH, 2], dt.float32, kind="ExternalOutput")
    if TAPS:
        tap_h = nc.dram_tensor("tap_h", [256, F], dt.float16, kind="ExternalOutput")
        tap_n = nc.dram_tensor("tap_n", [256, 128], dt.float16, kind="ExternalOutput")
        tap_own = nc.dram_tensor("tap_own", [256, 128], dt.float16, kind="ExternalOutput")
        tap_h1 = nc.dram_tensor("tap_h1", [NSH, F], dt.float16, kind="ExternalOutput")
        tap_t2 = nc.dram_tensor("tap_t2", [256, F], dt.float16, kind="ExternalOutput")
        tap_ag = nc.dram_tensor("tap_ag", [256, F], dt.float16, kind="ExternalOutput")
        tap_h2 = nc.dram_tensor("tap_h2", [NSH, C2], dt.float16, kind="ExternalOutput")
        tap_f1 = nc.dram_tensor("tap_f1", [NSH, FH], dt.float16, kind="ExternalOutput")

    with tile.TileContext(nc) as tc, ExitStack() as octx:
        # persistent pools
        dram = octx.enter_context(tc.tile_pool(name="dram", bufs=1, space="DRAM"))
        cpool = octx.enter_context(tc.tile_pool(name="const", bufs=1))
        stash = octx.enter_context(tc.tile_pool(name="stash", bufs=1))

        # DRAM tables
        t1h_lo = dram.tile([NLO, F], dt.float16)
        t1h_hi = dram.tile([NHI, F], dt.float16)
        t1n_lo = dram.tile([NLO, 128], dt.float16)
        t1n_hi = dram.tile([NHI, 128], dt.float16)
        t1n_own = dram.tile([NT * 128, 128], dt.float16)
        t2h_own = dram.tile([NSH, F], dt.float16)
        t2n_own = dram.tile([NT * 128, 128], dt.float16)
        t2h_all = dram.tile([N, F], dt.float16, addr_space="Shared")

        # constants in SBUF
        iota16 = cpool.tile([128, 128], dt.float16)
        nc.sync.dma_start(iota16[:], iota_in[:])
        ident16 = cpool.tile([128, 128], dt.float16)
        nc.sync.dma_start(ident16[:], ident_in[:])
        w1a_sb = cpool.tile([IN, FA], dt.float16)
        nc.sync.dma_start(w1a_sb[:], W1aug[:])
        w2a_sb = cpool.tile([128, 2, FA], dt.float16)
        nc.sync.dma_start(w2a_sb[:], W2aug[:])
        b1_sb = cpool.tile([128, F], dt.float32)
        nc.sync.dma_start(b1_sb[:], b1rep[:])
        b2_sb = cpool.tile([128, C2], dt.float32)
        nc.sync.dma_start(b2_sb[:], b2rep[:])
        f1b_sb = cpool.tile([128, FH], dt.float32)
        nc.sync.dma_start(f1b_sb[:], f1brep[:])
        f2b_sb = cpool.tile([128, 2], dt.float32)
        nc.sync.dma_start(f2b_sb[:], f2brep[:])
        ff1_sb = cpool.tile([C2, FH], dt.float16)
        nc.sync.dma_start(ff1_sb[:], ff1w16[:])
        ff2_sb = cpool.tile([FH, 2], dt.float16)
        nc.sync.dma_start(ff2_sb[:], ff2w16[:])
        a1s_sb = cpool.tile([128, F], dt.float16)
        nc.sync.dma_start(a1s_sb[:], a1srep_d[:])
        a2s_sb = cpool.tile([128, F], dt.float16)
        nc.sync.dma_start(a2s_sb[:], a2srep_d[:])

        # layer-1 hidden transposed, kept in SBUF for the layer-2 dense
        h1T = stash.tile([128, 2, NT, 128], dt.float16)
        out_stage = stash.tile([128, NT, 2], dt.float32)

        # ------------------------------------------------------------------
        # phase A: full replicated layer-1 dense -> T1 tables
        # ------------------------------------------------------------------
        with ExitStack() as ctx:
            xp = ctx.enter_context(tc.tile_pool(name="xp", bufs=2))
            pp = ctx.enter_context(tc.tile_pool(name="pp", bufs=4, space="PSUM"))
            sp = ctx.enter_context(tc.tile_pool(name="sp", bufs=2))

            G = 8
            groups = []
            m0 = 0
            while m0 < NTG:
                g = min(G, NTG - m0)
                groups.append((m0, g))
                m0 += g
            for (m0, g) in groups:
                xs = xp.tile([IN, G * 128], dt.float16, tag="xs")
                rows_t = min(g * 128, N - m0 * 128)
                nc.sync.dma_start(xs[:, 0:rows_t], xT16[:, m0 * 128:m0 * 128 + rows_t])
                hst = sp.tile([128, G, F], dt.float16, tag="hst")
                nst = sp.tile([128, G, 128], dt.float16, tag="nst")
                for j in range(g):
                    m = m0 + j
                    rows = min(128, N - m * 128)
                    ps = pp.tile([128, FA], dt.float32, tag="ps")
                    nc.tensor.matmul(ps[0:rows, :], xs[:, j * 128:j * 128 + rows],
                                     w1a_sb[:], start=True, stop=True)
                    nc.scalar.activation(hst[0:rows, j, :], ps[0:rows, 0:F], ACT.Copy)
                    nc.vector.tensor_copy(nst[0:rows, j, 0:8], ps[0:rows, F:FA])
                # group DMA to tables (handle a group straddling SPLIT)
                r0 = m0 * 128
                rows_t = min(g * 128, N - r0)
                if r0 + rows_t <= SPLIT:
                    dst_h, dst_n, rr = t1h_lo, t1n_lo, r0
                elif r0 >= SPLIT:
                    dst_h, dst_n, rr = t1h_hi, t1n_hi, r0 - SPLIT
                else:
                    dst_h = None
                if dst_h is not None:
                    _wr_rows(nc, dst_h, rr, rows_t, hst, F)
                    _wr_rows(nc, dst_n, rr, rows_t, nst, 128)
                else:
                    a = SPLIT - r0
                    _wr_rows(nc, t1h_lo, r0, a, hst, F)
                    _wr_rows(nc, t1n_lo, r0, a, nst, 128)
                    _wr_rows(nc, t1h_hi, 0, rows_t - a, hst, F, col0=a // 128)
                    _wr_rows(nc, t1n_hi, 0, rows_t - a, nst, 128, col0=a // 128)

        # ------------------------------------------------------------------
        # own-narrow table for layer 1 (gather own rows from t1n lo/hi)
        # ------------------------------------------------------------------
        if PH >= 2:
         with ExitStack() as ctx:
            op_ = ctx.enter_context(tc.tile_pool(name="op", bufs=1))
            il = op_.tile([128, NT * 8], dt.int16)
            nc.sync.dma_start(il[:], iol_d[:])
            ih = op_.tile([128, NT * 8], dt.int16)
            nc.sync.dma_start(ih[:], ioh_d[:])
            msk = op_.tile([128, NT], dt.float16)
            nc.sync.dma_start(msk[:], omask_d[:])
            glo = op_.tile([128, NT, 128], dt.float16)
            _gather_split(nc, lambda a, b: glo[:, a:b, :], t1n_lo[:], il, NT,
                          128, 0)
            ghi = op_.tile([128, NT, 128], dt.float16)
            _gather_split(nc, lambda a, b: ghi[:, a:b, :], t1n_hi[:], ih, NT,
                          128, 1)
            mrg = op_.tile([128, NT, 128], dt.float16)
            # mrg = ghi + (glo - ghi) * mask
            nc.vector.tensor_tensor(mrg[:], glo[:], ghi[:], op=OP.subtract)
            nc.vector.tensor_tensor(
                mrg[:], mrg[:],
                msk[:].unsqueeze(2).broadcast_to([128, NT, 128]), op=OP.mult)
            nc.vector.tensor_tensor(mrg[:], mrg[:], ghi[:], op=OP.add)
            nc.sync.dma_start(
                t1n_own[:].rearrange("(t p) c -> p t c", p=128), mrg[:])

        # ------------------------------------------------------------------
        # edge phases
        # ------------------------------------------------------------------
        def edge_phase(ctx, name, tab_lo, tab_hi, n_own, asrep_sb, evict):
            ep = ctx.enter_context(tc.tile_pool(name=name + "e", bufs=2))
            pp = ctx.enter_context(tc.tile_pool(name=name + "p", bufs=2, space="PSUM"))
            for t in range(NT):
                ct = sched.ct[t]
                if ct == 0:
                    continue
                nlo, nhi = sched.n_lo[t], sched.n_hi[t]
                b0 = sched.base[t]
                ixh = ep.tile([128, ct * 8], dt.int16, tag="ixh")
                nc.sync.dma_start(ixh[:], idx_h_d[:, b0 * 8:(b0 + ct) * 8])
                ixn = ep.tile([128, ct * 8], dt.int16, tag="ixn")
                nc.sync.dma_start(ixn[:], idx_nd_d[:, b0 * 8:(b0 + ct) * 8])
                dl = ep.tile([128, ct], dt.float32, tag="dl")
                nc.sync.dma_start(dl[:], dstloc_d[:, b0:b0 + ct])

                # local-table gather + one-hot build first: at the layer-2
                # boundary these depend only on phase-C outputs, so they can
                # proceed while the AllGather (which gates the g gathers
                # below) is still in flight.
                g = ep.tile([128, ct, F], dt.float16, tag="g")
                nd_ = ep.tile([128, ct, 128], dt.float16, tag="nd")
                _gather_split(nc, lambda a, b: nd_[:, a:b, :], n_own, ixn,
                              ct, 128, 1)
                oh = ep.tile([128, ct, 128], dt.float16, tag="oh")
                nc.vector.tensor_tensor(
                    oh[:],
                    iota16[:].unsqueeze(1).broadcast_to([128, ct, 128]),
                    dl[:].unsqueeze(2).broadcast_to([128, ct, 128]),
                    op=OP.is_equal)
                if nlo:
                    _gather_split(nc, lambda a, b: g[:, a:b, :], tab_lo,
                                  ixh, nlo, F, 0)
                if nhi:
                    _gather_split(
                        nc, lambda a, b: g[:, nlo + a:nlo + b, :], tab_hi,
                        ixh[:, nlo * 8:ct * 8], nhi, F, 2)

                # a_src per edge from the gathered rows: ns[e,h] =
                # sum_c g[e,h*C+c] * att_src[h,c]; rhs[:, :, 0:F] is scratch
                # for the product (overwritten later by the fold).
                rhs = ep.tile([128, ct, F + H], dt.float16, tag="rhs")
                nc.vector.tensor_tensor(
                    rhs[:, :, 0:F], g[:],
                    asrep_sb[:].unsqueeze(1).broadcast_to([128, ct, F]),
                    op=OP.mult)
                nsr = ep.tile([128, ct, H], dt.float32, tag="nsr")
                nc.vector.tensor_reduce(
                    nsr[:], rhs[:, :, 0:F].rearrange("p c (h d) -> p c h d",
                                                     h=H),
                    axis=mybir.AxisListType.X, op=OP.add)

                # narrow: alpha = lrelu(a_src + a_dst); ex = exp(alpha)
                alpha = ep.tile([128, ct, H], dt.float32, tag="alpha")
                nc.vector.tensor_tensor(alpha[:], nsr[:], nd_[:, :, H:2 * H],
                                        op=OP.add)
                nc.vector.scalar_tensor_tensor(
                    alpha[:], alpha[:], float(NEG_SLOPE), alpha[:],
                    op0=OP.mult, op1=OP.max)
                nc.scalar.activation(rhs[:, :, F:F + H], alpha[:], ACT.Exp)
                # fold: rhs[:, :, 0:F] = g * ex (per-head broadcast)
                nc.vector.tensor_tensor(
                    rhs[:, :, 0:F].rearrange("p c (h d) -> p c h d", h=H),
                    g[:].rearrange("p c (h d) -> p c h d", h=H),
                    rhs[:, :, F:F + H].unsqueeze(3).broadcast_to(
                        [128, ct, H, F // H]),
                    op=OP.mult)
                # matmul-scatter (one-hot built above, before the g gathers)
                ps = pp.tile([128, F + H], dt.float32, tag="ps")
                for c in range(ct):
                    nc.tensor.matmul(ps[:], oh[:, c, :], rhs[:, c, :],
                                     start=(c == 0), stop=(c == ct - 1))
                evict(ep, pp, t, ps)

        # ---- layer 1 evict: h1 = relu(agg/den + b1); build h1T + h1own ----
        def evict1(ep, pp, t, ps):
            rows = min(128, NSH - t * 128)
            rcp = ep.tile([128, H], dt.float32, tag="rcp")
            nc.vector.reciprocal(rcp[:], ps[:, F:F + H])
            pre = ep.tile([128, F], dt.float32, tag="pre")
            nc.vector.tensor_tensor(
                pre[:].rearrange("p (h d) -> p h d", h=H),
                ps[:, 0:F].rearrange("p (h d) -> p h d", h=H),
                rcp[:].unsqueeze(2).broadcast_to([128, H, F // H]), op=OP.mult)
            nc.vector.tensor_tensor(pre[:], pre[:], b1_sb[:], op=OP.add)
            h1r = ep.tile([128, F], dt.float16, tag="h1r")
            nc.scalar.activation(h1r[:], pre[:], ACT.Relu)
            if TAPS:
                nc.sync.dma_start(tap_h1[t * 128:t * 128 + rows, :], h1r[0:rows, :])
            for b in range(2):
                tp = pp.tile([128, 128], dt.float16, tag="tp")
                nc.tensor.transpose(tp[:], h1r[:, b * 128:(b + 1) * 128], ident16[:])
                nc.scalar.activation(h1T[:, b, t, :], tp[:], ACT.Copy)
            # fused layer-2 dense for this tile: t2h_own rows are complete as
            # soon as the L1 edge phase finishes, so the AllGather can launch
            # without a separate dense pass in between.
            ps2 = pp.tile([128, FA], dt.float32, tag="ps2")
            for b in range(2):
                nc.tensor.matmul(ps2[:], h1T[:, b, t, :], w2a_sb[:, b, :],
                                 start=(b == 0), stop=(b == 1))
            hst = ep.tile([128, F], dt.float16, tag="hst")
            nc.scalar.activation(hst[:], ps2[:, 0:F], ACT.Copy)
            nst = ep.tile([128, 128], dt.float16, tag="nst")
            nc.vector.tensor_copy(nst[:, 0:8], ps2[:, F:FA])
            nc.sync.dma_start(t2h_own[t * 128:t * 128 + rows, :],
                              hst[0:rows, :])
            nc.sync.dma_start(t2n_own[t * 128:(t + 1) * 128, :], nst[:])

        if PH >= 3:
         with ExitStack() as ctx:
            edge_phase(ctx, "l1", t1h_lo[:], t1h_hi[:], t1n_own[:], a1s_sb,
                       evict1)

        # ------------------------------------------------------------------
        # (layer-2 dense is fused into evict1 above)
        # ------------------------------------------------------------------
        # phase D: exchange (h only; per-edge a_src is recomputed from the
        # gathered rows, so no narrow-table exchange is needed)
        # ------------------------------------------------------------------
        if PH >= 5:
         nc.gpsimd.collective_compute(
            "AllGather", OP.bypass, replica_groups=[list(range(NCORES))],
            ins=[t2h_own[:].opt()], outs=[t2h_all[:].opt()])

        # ---- layer 2 evict: h2 = relu(mean_h(agg/den) + b2); FF head ----
        def evict2(ep, pp, t, ps):
            rows = min(128, NSH - t * 128)
            rcp = ep.tile([128, H], dt.float32, tag="rcp")
            nc.vector.reciprocal(rcp[:], ps[:, F:F + H])
            pre = ep.tile([128, H, C2], dt.float32, tag="pre")
            nc.vector.tensor_tensor(
                pre[:], ps[:, 0:F].rearrange("p (h d) -> p h d", h=H),
                rcp[:].unsqueeze(2).broadcast_to([128, H, C2]), op=OP.mult)
            red = ep.tile([128, C2], dt.float32, tag="red")
            nc.vector.tensor_reduce(red[:], pre[:].transpose([0, 2, 1]),
                                    axis=mybir.AxisListType.X, op=OP.add)
            nc.vector.scalar_tensor_tensor(red[:], red[:], 1.0 / H, b2_sb[:],
                                           op0=OP.mult, op1=OP.add)
            h2 = ep.tile([128, 128], dt.float16, tag="h2")
            nc.vector.memset(h2[:, C2:128], 0.0)
            nc.scalar.activation(h2[:, 0:C2], red[:], ACT.Relu)
            if TAPS:
                nc.sync.dma_start(tap_h2[t * 128:t * 128 + rows, :],
                                  h2[0:rows, 0:C2])
            # FF: out = relu(h2 @ ff1 + b1f) @ ff2 + b2f  (square transposes)
            tp = pp.tile([128, 128], dt.float16, tag="tp2", bufs=1)
            nc.tensor.transpose(tp[:], h2[:], ident16[:])
            h2T = ep.tile([C2, 128], dt.float16, tag="h2T")
            nc.scalar.activation(h2T[:], tp[0:C2, :], ACT.Copy)
            pf1 = pp.tile([128, FH], dt.float32, tag="pf1", bufs=1)
            nc.tensor.matmul(pf1[:], h2T[:], ff1_sb[:], start=True, stop=True)
            f1p = ep.tile([128, FH], dt.float32, tag="f1p")
            nc.vector.tensor_tensor(f1p[:], pf1[:], f1b_sb[:], op=OP.add)
            f1 = ep.tile([128, 128], dt.float16, tag="f1")
            nc.vector.memset(f1[:, FH:128], 0.0)
            nc.scalar.activation(f1[:, 0:FH], f1p[:], ACT.Relu)
            if TAPS:
                nc.sync.dma_start(tap_f1[t * 128:t * 128 + rows, :],
                                  f1[0:rows, 0:FH])
            tpf = pp.tile([128, 128], dt.float16, tag="tpf", bufs=1)
            nc.tensor.transpose(tpf[:], f1[:], ident16[:])
            f1T = ep.tile([FH, 128], dt.float16, tag="f1T")
            nc.scalar.activation(f1T[:], tpf[0:FH, :], ACT.Copy)
            pf2 = pp.tile([128, 2], dt.float32, tag="pf2", bufs=1)
            nc.tensor.matmul(pf2[:], f1T[:], ff2_sb[:], start=True, stop=True)
            nc.vector.tensor_tensor(out_stage[:, t, :], pf2[:], f2b_sb[:],
                                    op=OP.add)

        if TAPS:
            A = SPLIT - 128
            nc.sync.dma_start(tap_h[0:128, :], t1h_lo[A:A + 128, :])
            nc.sync.dma_start(tap_h[128:256, :], t1h_hi[0:128, :])
            nc.sync.dma_start(tap_n[0:128, :], t1n_lo[A:A + 128, :])
            nc.sync.dma_start(tap_n[128:256, :], t1n_hi[0:128, :])
            nc.sync.dma_start(tap_own[:], t1n_own[0:256, :])
            nc.sync.dma_start(tap_t2[:], t2h_own[0:256, :])
            nc.sync.dma_start(tap_ag[:], t2h_all[NSH:NSH + 256, :])
        if PH >= 6:
         with ExitStack() as ctx:
            edge_phase(ctx, "l2", t2h_all[0:SPLIT, :], t2h_all[SPLIT:N, :],
                       t2n_own[:], a2s_sb, evict2)

        # final output
        if PH < 6:
            nc.vector.memset(out_stage[:], 0.0)
        full = (NSH // 128) * 128
        if full:
            nc.sync.dma_start(
                out_d[0:full, :].rearrange("(t p) j -> p t j", p=128),
                out_stage[:, 0:full // 128, :])
        if NSH > full:
            nc.sync.dma_start(out_d[full:NSH, :],
                              out_stage[0:NSH - full, NT - 1, :])

    nc.compile()
    return nc


def _wr_rows(nc, dst, r0, rows, st, width, col0=0):
    """DMA staging [128, G, width] (rows r = g*128+p at [p, g]) to DRAM rows
    dst[r0:r0+rows]. col0: starting tile index inside the staging buffer."""
    g_full = rows // 128
    if g_full:
        nc.sync.dma_start(
            dst[r0:r0 + g_full * 128, :].rearrange("(g p) c -> p g c", p=128),
            st[:, col0:col0 + g_full, :])
    rem = rows - g_full * 128
    if rem:
        nc.sync.dma_start(dst[r0 + g_full * 128:r0 + rows, :],
                          st[0:rem, col0 + g_full, :])


# ----------------------------------------------------------------------------
# entry point
# ----------------------------------------------------------------------------

_CACHE = {}
_RUNNER_CACHE = {}


def _make_runner(nc):
    """Persistent jitted shard_map runner for nc (mirrors
    bass2jax.run_bass_via_pjrt but caches the traced computation so repeat
    kernel() calls skip retrace/recompile; inputs are uploaded per call)."""
    import jax
    import concourse.mybir as mybir_
    from concourse.bass2jax import _bass_exec_p, partition_id_tensor, \
        install_neuronx_cc_hook
    from jax.sharding import Mesh, PartitionSpec, NamedSharding
    from jax.experimental.shard_map import shard_map

    install_neuronx_cc_hook()
    partition_name = (nc.partition_id_tensor.name
                      if nc.partition_id_tensor else None)
    in_names, out_names, out_avals, zero_outs = [], [], [], []
    for alloc in nc.m.functions[0].allocations:
        if not isinstance(alloc, mybir_.MemoryLocationSet):
            continue
        name = alloc.memorylocations[0].name
        if alloc.kind == "ExternalInput":
            if name != partition_name:
                in_names.append(name)
        elif alloc.kind == "ExternalOutput":
            shape = tuple(alloc.tensor_shape)
            dtype = mybir_.dt.np(alloc.dtype)
            out_names.append(name)
            out_avals.append(jax.core.ShapedArray(shape, dtype))
            zero_outs.append(np.zeros(shape, dtype))
    n_params = len(in_names)
    n_outs = len(out_avals)
    all_in = list(in_names) + list(out_names)
    if partition_name is not None:
        all_in.append(partition_name)
    donate = tuple(range(n_params, n_params + n_outs))

    def _body(*args):
        operands = list(args)
        if partition_name is not None:
            operands.append(partition_id_tensor())
        return tuple(_bass_exec_p.bind(
            *operands, out_avals=tuple(out_avals), in_names=tuple(all_in),
            out_names=tuple(out_names), lowering_input_output_aliases=(),
            sim_require_finite=True, sim_require_nnan=True, nc=nc))

    devices = jax.devices()[:NCORES]
    mesh = Mesh(np.asarray(devices), ("core",))
    sharded = jax.jit(
        shard_map(_body, mesh=mesh,
                  in_specs=(PartitionSpec("core"),) * (n_params + n_outs),
                  out_specs=(PartitionSpec("core"),) * n_outs,
                  check_rep=False),
        donate_argnums=donate, keep_unused=True)
    sh = NamedSharding(mesh, PartitionSpec("core"))

    def run(in_maps):
        concat_in = [
            jax.device_put(np.concatenate(
                [np.asarray(in_maps[c][n]) for c in range(NCORES)], axis=0),
                sh)
            for n in in_names]
        zs = [jax.device_put(
            np.zeros((NCORES * z.shape[0], *z.shape[1:]), z.dtype), sh)
            for z in zero_outs]
        outs = sharded(*concat_in, *zs)
        return [{name: np.asarray(outs[i]).reshape(
                    NCORES, *out_avals[i].shape)[c]
                 for i, name in enumerate(out_names)}
                for c in range(NCORES)]

    return run


def kernel(x, edge_index, edge_attr, W1, att_src1, att_dst1, b1,
           W2, att_src2, att_dst2, b2, ff1_w, ff1_b, ff2_w, ff2_b):
    x = np.asarray(x, np.float32)
    edge_index = np.asarray(edge_index)
    args = [np.asarray(a, np.float32) for a in
            (W1, att_src1, att_dst1, b1, W2, att_src2, att_dst2, b2,
             ff1_w, ff1_b, ff2_w, ff2_b)]
    in_maps, sched, dims = _prep(x, edge_index, *args)
    key = (dims["N"], dims["IN"], tuple(sched.n_lo), tuple(sched.n_hi))
    if key not in _CACHE:
        nc_built = _build(sched, dims)
        _CACHE[key] = (nc_built, dims["salt_name"])
    nc, salt_name = _CACHE[key]
    salt = np.zeros((1, 4), np.float32)
    for m in in_maps:
        m[salt_name] = salt
    if key not in _RUNNER_CACHE:
        _RUNNER_CACHE[key] = _make_runner(nc)
    res = _RUNNER_CACHE[key](in_maps)
    out = np.concatenate([res[k]["out"] for k in range(NCORES)], axis=0)
    return out.astype(np.float32)



# revision 30
# speedup vs baseline: 1.1501x; 1.0607x over previous
"""GAT (2-layer GATConv + FF head) on 8 Trainium2 NeuronCores.

Strategy (per sharding hint): nodes + incident edges partitioned by
destination across 8 cores; per-edge softmax/scatter local to the
destination shard via one-hot matmul-scatter into PSUM; small weights
replicated. Layer-1 node features are computed fully replicated (input x
is available everywhere); layer-2 features are computed on the owning
shard and exchanged with a single feature AllGather. Random-access reads
(h[src] rows, own-shard a_dst rows) use SWDGE dma_gather from fp16 DRAM
tables. Per-edge a_src is recomputed on the fly from the gathered rows
(dot with att_src on VectorE), so no global narrow tables or narrow
exchange exist. The layer-2 dense is fused into the layer-1 edge-phase
evict so t2h_own completes with the edge phase and the AllGather fires
immediately; each tile's local nd gather + one-hot build are issued
before its table gathers so local work overlaps the collective.

Message path in fp16 (tables, gathered rows, attention weights, matmul
operands); accumulation in fp32 PSUM; per-edge logits in fp32.
"""
import sys
sys.path.insert(0, "/opt/trn_rl_repo")

import numpy as np
from contextlib import ExitStack

import concourse.bass as bass
import concourse.bacc as bacc
import concourse.tile as tile
import concourse.mybir as mybir
from concourse.bass_utils import run_bass_kernel_spmd

dt = mybir.dt
OP = mybir.AluOpType
ACT = mybir.ActivationFunctionType

NCORES = 8
H = 4
NEG_SLOPE = 0.2


# ----------------------------------------------------------------------------
# host-side prep
# ----------------------------------------------------------------------------

def _wrap_idx(idx):
    """Pack an index list into the SWDGE wrapped layout [128, n/16] int16:
    index i -> partition i%16 (replicated to all 8 16-partition groups),
    free offset i//16."""
    n = len(idx)
    assert n % 128 == 0
    out = np.zeros((128, n // 16), np.int16)
    a = np.asarray(idx, np.int16).reshape(n // 16, 16).T  # [16, n/16]
    for r in range(8):
        out[r * 16:(r + 1) * 16, :] = a
    return out


def _pad128(a, fill):
    n = len(a)
    m = ((n + 127) // 128) * 128
    return np.concatenate([a, np.full(m - n, fill, a.dtype)])


class Sched:
    """Static, core-uniform per-tile chunk schedule."""

    def __init__(self, n_lo, n_hi):
        self.n_lo = n_lo          # [NT] chunks for lo-half gathers
        self.n_hi = n_hi          # [NT] chunks for hi-half gathers
        self.ct = [a + b for a, b in zip(n_lo, n_hi)]
        self.base = np.concatenate([[0], np.cumsum(self.ct)]).astype(int)
        self.total = int(self.base[-1])  # total chunks per core


_EDGE_CACHE = {}


def _prep(x, edge_index, W1, att_src1, att_dst1, b1, W2, att_src2, att_dst2,
          b2, ff1_w, ff1_b, ff2_w, ff2_b):
    N, IN = x.shape
    F = W1.shape[1]               # 256
    C1 = F // H
    C2 = W2.shape[1] // H
    NSH = N // NCORES
    NT = (NSH + 127) // 128
    NLO_T = (N // 2) // 128
    SPLIT = NLO_T * 128

    import hashlib as _hl
    ekey = (N, _hl.sha1(np.ascontiguousarray(edge_index).tobytes())
            .hexdigest())
    cached = _EDGE_CACHE.get(ekey)
    if cached is not None:
        (sched, idx_h, idx_nd, dstloc, idx_own_lo, idx_own_hi,
         own_mask) = cached
        return _prep_finish(x, W1, att_src1, att_dst1, b1, W2, att_src2,
                            att_dst2, b2, ff1_w, ff1_b, ff2_w, ff2_b,
                            sched, idx_h, idx_nd, dstloc, idx_own_lo,
                            idx_own_hi, own_mask,
                            N, IN, F, NSH, NT, SPLIT, C1, C2)

    E = edge_index.shape[1]
    ar = np.arange(N, dtype=np.int64)
    src = np.concatenate([edge_index[0], ar])
    dst = np.concatenate([edge_index[1], ar])

    shard = dst // NSH
    dstloc_all = dst - shard * NSH

    # group edges per (core, tile, half)
    per = [[[None, None] for _ in range(NT)] for _ in range(NCORES)]
    for k in range(NCORES):
        m = shard == k
        s_k, dl_k = src[m], dstloc_all[m]
        t_k = dl_k // 128
        for t in range(NT):
            mt = t_k == t
            s_t, dl_t = s_k[mt], dl_k[mt]
            lo = s_t < SPLIT
            per[k][t][0] = (s_t[lo], dl_t[lo])
            per[k][t][1] = (s_t[~lo] - SPLIT, dl_t[~lo])

    n_lo = [max((len(per[k][t][0][0]) + 127) // 128 for k in range(NCORES))
            for t in range(NT)]
    n_hi = [max((len(per[k][t][1][0]) + 127) // 128 for k in range(NCORES))
            for t in range(NT)]
    sched = Sched(n_lo, n_hi)

    # per-core edge arrays in schedule order
    idx_h = []       # [128, total*8] int16  (gather idx, lo/hi-local rows)
    idx_nd = []      # [128, total*8] int16  (dstlocal shard rows)
    dstloc = []      # [128, total] f32      (tile-local dst or -1)
    for k in range(NCORES):
        ih = np.zeros((128, sched.total * 8), np.int16)
        nd = np.zeros((128, sched.total * 8), np.int16)
        dl = np.full((128, sched.total), -1.0, np.float32)
        for t in range(NT):
            off = sched.base[t]
            for half, nch in ((0, n_lo[t]), (1, n_hi[t])):
                if nch == 0:
                    continue
                s_t, dl_t = per[k][t][half]
                ne = nch * 128
                sp = _pad128(np.concatenate([s_t, np.zeros(ne - len(s_t),
                                                           np.int64)]), 0)[:ne]
                sp[len(s_t):] = 0
                dlp = np.full(ne, -1.0, np.float32)
                dlp[:len(dl_t)] = (dl_t - t * 128).astype(np.float32)
                ndp = np.zeros(ne, np.int64)
                ndp[:len(dl_t)] = dl_t
                ih[:, off * 8:(off + nch) * 8] = _wrap_idx(sp)
                nd[:, off * 8:(off + nch) * 8] = _wrap_idx(ndp)
                dl[:, off:off + nch] = dlp.reshape(nch, 128).T
                off += nch
        idx_h.append(ih)
        idx_nd.append(nd)
        dstloc.append(dl)

    # own-narrow build: gather T1n rows for own shard (lo/hi + select mask)
    own_rows_pad = NT * 128
    idx_own_lo, idx_own_hi, own_mask = [], [], []
    for k in range(NCORES):
        rows = np.arange(k * NSH, (k + 1) * NSH)
        rows = np.concatenate([rows, np.full(own_rows_pad - NSH, rows[0])])
        is_lo = rows < SPLIT
        lo_i = np.where(is_lo, rows, 0)
        hi_i = np.where(is_lo, 0, rows - SPLIT)
        idx_own_lo.append(_wrap_idx(lo_i))
        idx_own_hi.append(_wrap_idx(hi_i))
        m = np.zeros((128, NT), np.float16)
        m[:, :] = is_lo.reshape(NT, 128).T.astype(np.float16)
        own_mask.append(m)

    _EDGE_CACHE[ekey] = (sched, idx_h, idx_nd, dstloc, idx_own_lo,
                         idx_own_hi, own_mask)
    return _prep_finish(x, W1, att_src1, att_dst1, b1, W2, att_src2,
                        att_dst2, b2, ff1_w, ff1_b, ff2_w, ff2_b,
                        sched, idx_h, idx_nd, dstloc, idx_own_lo,
                        idx_own_hi, own_mask,
                        N, IN, F, NSH, NT, SPLIT, C1, C2)


def _prep_finish(x, W1, att_src1, att_dst1, b1, W2, att_src2, att_dst2, b2,
                 ff1_w, ff1_b, ff2_w, ff2_b, sched, idx_h, idx_nd, dstloc,
                 idx_own_lo, idx_own_hi, own_mask,
                 N, IN, F, NSH, NT, SPLIT, C1, C2):
    # weights
    def aug(W, a_s, a_d, C):
        v_s = np.einsum("fhc,hc->fh", W.reshape(-1, H, C), a_s)
        v_d = np.einsum("fhc,hc->fh", W.reshape(-1, H, C), a_d)
        return np.concatenate([W, v_s, v_d], axis=1).astype(np.float16)

    W1aug = aug(W1, att_src1, att_dst1, C1)              # [IN, F+8]
    W2aug = aug(W2, att_src2, att_dst2, C2)              # [F, F+8]
    W2aug_pk = W2aug.reshape(2, 128, F + 8).transpose(1, 0, 2).copy()

    xT16 = np.ascontiguousarray(x.T).astype(np.float16)  # [IN, N]

    iota16 = np.tile(np.arange(128, dtype=np.float16), (128, 1))
    ident16 = np.eye(128, dtype=np.float16)

    const = {
        "xT16": xT16, "W1aug": W1aug, "W2aug": W2aug_pk,
        "iota16": iota16, "ident16": ident16,
        "a1srep": np.tile(att_src1.astype(np.float16).reshape(1, F),
                          (128, 1)),
        "a2srep": np.tile(att_src2.astype(np.float16).reshape(1, F),
                          (128, 1)),
        "b1rep": np.tile(b1.astype(np.float32), (128, 1)),
        "b2rep": np.tile(b2.astype(np.float32), (128, 1)),
        "f1brep": np.tile(ff1_b.astype(np.float32), (128, 1)),
        "f2brep": np.tile(ff2_b.astype(np.float32), (128, 1)),
        "ff1w16": ff1_w.astype(np.float16),
        "ff2w16": ff2_w.astype(np.float16),
    }

    in_maps = []
    for k in range(NCORES):
        m = dict(const)
        m.update({
            "idx_h": idx_h[k], "idx_nd": idx_nd[k], "dstloc": dstloc[k],
            "idx_own_lo": idx_own_lo[k], "idx_own_hi": idx_own_hi[k],
            "own_mask": own_mask[k],
        })
        in_maps.append(m)

    dims = dict(N=N, IN=IN, F=F, NSH=NSH, NT=NT, SPLIT=SPLIT,
                NLO=SPLIT, NHI=N - SPLIT, C2=C2, FH=ff1_w.shape[1])
    return in_maps, sched, dims


# ----------------------------------------------------------------------------
# device program
# ----------------------------------------------------------------------------

def _gather_split(nc, out_ap_fn, tab, idx_sb, n_chunks, elem, q0):
    """Emit dma_gather calls capped at 8 chunks (1024 idxs) each.
    out_ap_fn(c0, c1) -> output AP for chunk range; idx_sb indexed per chunk."""
    c0 = 0
    q = q0
    while c0 < n_chunks:
        c1 = min(c0 + 8, n_chunks)
        nc.gpsimd.dma_gather(
            out_ap_fn(c0, c1), tab, idx_sb[:, c0 * 8:c1 * 8],
            num_idxs=(c1 - c0) * 128, num_idxs_reg=(c1 - c0) * 128,
            elem_size=elem, queue_num=q % 4)
        q += 1
        c0 = c1


def _build(sched, dims):
    import os, hashlib
    PH = int(os.environ.get('K_PHASES', '6'))
    TAPS = int(os.environ.get('K_TAPS', '0'))
    N, IN, F, NSH, NT = dims["N"], dims["IN"], dims["F"], dims["NSH"], dims["NT"]
    NLO, NHI, SPLIT = dims["NLO"], dims["NHI"], dims["SPLIT"]
    C2, FH = dims["C2"], dims["FH"]
    FA = F + 8
    NTG = (N + 127) // 128          # global node tiles
    NTG_LO = NLO // 128

    nc = bacc.Bacc("TRN2", target_bir_lowering=False, num_devices=NCORES,
                   num_swdge_queues=4)
    # The neuronx compile cache keys on the jit signature only (the embedded
    # BIR is not hashed), so two different programs with identical I/O would
    # alias to one NEFF. A content-named dummy input de-aliases them.
    with open(__file__, "rb") as _f:
        _salt = hashlib.sha256(
            _f.read() + repr((sched.n_lo, sched.n_hi, sorted(dims.items()),
                              PH)).encode()).hexdigest()[:16]
    nc.dram_tensor(f"salt_{_salt}", [1, 4], dt.float32, kind="ExternalInput")
    dims["salt_name"] = f"salt_{_salt}"

    # inputs
    xT16 = nc.dram_tensor("xT16", [IN, N], dt.float16, kind="ExternalInput")
    W1aug = nc.dram_tensor("W1aug", [IN, FA], dt.float16, kind="ExternalInput")
    W2aug = nc.dram_tensor("W2aug", [128, 2, FA], dt.float16, kind="ExternalInput")
    iota_in = nc.dram_tensor("iota16", [128, 128], dt.float16, kind="ExternalInput")
    ident_in = nc.dram_tensor("ident16", [128, 128], dt.float16, kind="ExternalInput")
    b1rep = nc.dram_tensor("b1rep", [128, F], dt.float32, kind="ExternalInput")
    b2rep = nc.dram_tensor("b2rep", [128, C2], dt.float32, kind="ExternalInput")
    f1brep = nc.dram_tensor("f1brep", [128, FH], dt.float32, kind="ExternalInput")
    f2brep = nc.dram_tensor("f2brep", [128, 2], dt.float32, kind="ExternalInput")
    ff1w16 = nc.dram_tensor("ff1w16", [C2, FH], dt.float16, kind="ExternalInput")
    ff2w16 = nc.dram_tensor("ff2w16", [FH, 2], dt.float16, kind="ExternalInput")
    a1srep_d = nc.dram_tensor("a1srep", [128, F], dt.float16, kind="ExternalInput")
    a2srep_d = nc.dram_tensor("a2srep", [128, F], dt.float16, kind="ExternalInput")
    idx_h_d = nc.dram_tensor("idx_h", [128, sched.total * 8], dt.int16, kind="ExternalInput")
    idx_nd_d = nc.dram_tensor("idx_nd", [128, sched.total * 8], dt.int16, kind="ExternalInput")
    dstloc_d = nc.dram_tensor("dstloc", [128, sched.total], dt.float32, kind="ExternalInput")
    iol_d = nc.dram_tensor("idx_own_lo", [128, NT * 8], dt.int16, kind="ExternalInput")
    ioh_d = nc.dram_tensor("idx_own_hi", [128, NT * 8], dt.int16, kind="ExternalInput")
    omask_d = nc.dram_tensor("own_mask", [128, NT], dt.float16, kind="ExternalInput")

    out_d = nc.dram_tensor("out", [NSH, 2], dt.float32, kind="ExternalOutput")
    if TAPS:
        tap_h = nc.dram_tensor("tap_h", [256, F], dt.float16, kind="ExternalOutput")
        tap_n = nc.dram_tensor("tap_n", [256, 128], dt.float16, kind="ExternalOutput")
        tap_own = nc.dram_tensor("tap_own", [256, 128], dt.float16, kind="ExternalOutput")
        tap_h1 = nc.dram_tensor("tap_h1", [NSH, F], dt.float16, kind="ExternalOutput")
        tap_t2 = nc.dram_tensor("tap_t2", [256, F], dt.float16, kind="ExternalOutput")
        tap_ag = nc.dram_tensor("tap_ag", [256, F], dt.float16, kind="ExternalOutput")
        tap_h2 = nc.dram_tensor("tap_h2", [NSH, C2], dt.float16, kind="ExternalOutput")
        tap_f1 = nc.dram_tensor("tap_f1", [NSH, FH], dt.float16, kind="ExternalOutput")

    with tile.TileContext(nc) as tc, ExitStack() as octx:
        # persistent pools
        dram = octx.enter_context(tc.tile_pool(name="dram", bufs=1, space="DRAM"))
        cpool = octx.enter_context(tc.tile_pool(name="const", bufs=1))
        stash = octx.enter_context(tc.tile_pool(name="stash", bufs=1))

        # DRAM tables
        t1h_lo = dram.tile([NLO, F], dt.float16)
        t1h_hi = dram.tile([NHI, F], dt.float16)
        t1n_lo = dram.tile([NLO, 128], dt.float16)
        t1n_hi = dram.tile([NHI, 128], dt.float16)
        t1n_own = dram.tile([NT * 128, 128], dt.float16)
        t2h_own = dram.tile([NSH, F], dt.float16)
        t2n_own = dram.tile([NT * 128, 128], dt.float16)
        t2h_all = dram.tile([N, F], dt.float16, addr_space="Shared")

        # constants in SBUF
        iota16 = cpool.tile([128, 128], dt.float16)
        nc.sync.dma_start(iota16[:], iota_in[:])
        ident16 = cpool.tile([128, 128], dt.float16)
        nc.sync.dma_start(ident16[:], ident_in[:])
        w1a_sb = cpool.tile([IN, FA], dt.float16)
        nc.sync.dma_start(w1a_sb[:], W1aug[:])
        w2a_sb = cpool.tile([128, 2, FA], dt.float16)
        nc.sync.dma_start(w2a_sb[:], W2aug[:])
        b1_sb = cpool.tile([128, F], dt.float32)
        nc.sync.dma_start(b1_sb[:], b1rep[:])
        b2_sb = cpool.tile([128, C2], dt.float32)
        nc.sync.dma_start(b2_sb[:], b2rep[:])
        f1b_sb = cpool.tile([128, FH], dt.float32)
        nc.sync.dma_start(f1b_sb[:], f1brep[:])
        f2b_sb = cpool.tile([128, 2], dt.float32)
        nc.sync.dma_start(f2b_sb[:], f2brep[:])
        ff1_sb = cpool.tile([C2, FH], dt.float16)
        nc.sync.dma_start(ff1_sb[:], ff1w16[:])
        ff2_sb = cpool.tile([FH, 2], dt.float16)
        nc.sync.dma_start(ff2_sb[:], ff2w16[:])
        a1s_sb = cpool.tile([128, F], dt.float16)
        nc.sync.dma_start(a1s_sb[:], a1srep_d[:])
        a2s_sb = cpool.tile([128, F], dt.float16)
        nc.sync.dma_start(a2s_sb[:], a2srep_d[:])

        # layer-1 hidden transposed, kept in SBUF for the layer-2 dense
        h1T = stash.tile([128, 2, NT, 128], dt.float16)
        out_stage = stash.tile([128, NT, 2], dt.float32)

        # ------------------------------------------------------------------
        # phase A: full replicated layer-1 dense -> T1 tables
        # ------------------------------------------------------------------
        with ExitStack() as ctx:
            xp = ctx.enter_context(tc.tile_pool(name="xp", bufs=2))
            pp = ctx.enter_context(tc.tile_pool(name="pp", bufs=4, space="PSUM"))
            sp = ctx.enter_context(tc.tile_pool(name="sp", bufs=2))

            G = 8
            groups = []
            m0 = 0
            while m0 < NTG:
                g = min(G, NTG - m0)
                groups.append((m0, g))
                m0 += g
            for (m0, g) in groups:
                xs = xp.tile([IN, G * 128], dt.float16, tag="xs")
                rows_t = min(g * 128, N - m0 * 128)
                nc.sync.dma_start(xs[:, 0:rows_t], xT16[:, m0 * 128:m0 * 128 + rows_t])
                hst = sp.tile([128, G, F], dt.float16, tag="hst")
                nst = sp.tile([128, G, 128], dt.float16, tag="nst")
                for j in range(g):
                    m = m0 + j
                    rows = min(128, N - m * 128)
                    ps = pp.tile([128, FA], dt.float32, tag="ps")
                    nc.tensor.matmul(ps[0:rows, :], xs[:, j * 128:j * 128 + rows],
                                     w1a_sb[:], start=True, stop=True)
                    nc.scalar.activation(hst[0:rows, j, :], ps[0:rows, 0:F], ACT.Copy)
                    nc.vector.tensor_copy(nst[0:rows, j, 0:8], ps[0:rows, F:FA])
                # group DMA to tables (handle a group straddling SPLIT)
                r0 = m0 * 128
                rows_t = min(g * 128, N - r0)
                if r0 + rows_t <= SPLIT:
                    dst_h, dst_n, rr = t1h_lo, t1n_lo, r0
                elif r0 >= SPLIT:
                    dst_h, dst_n, rr = t1h_hi, t1n_hi, r0 - SPLIT
                else:
                    dst_h = None
                if dst_h is not None:
                    _wr_rows(nc, dst_h, rr, rows_t, hst, F)
                    _wr_rows(nc, dst_n, rr, rows_t, nst, 128)
                else:
                    a = SPLIT - r0
                    _wr_rows(nc, t1h_lo, r0, a, hst, F)
                    _wr_rows(nc, t1n_lo, r0, a, nst, 128)
                    _wr_rows(nc, t1h_hi, 0, rows_t - a, hst, F, col0=a // 128)
                    _wr_rows(nc, t1n_hi, 0, rows_t - a, nst, 128, col0=a // 128)

        # ------------------------------------------------------------------
        # own-narrow table for layer 1 (gather own rows from t1n lo/hi)
        # ------------------------------------------------------------------
        if PH >= 2:
         with ExitStack() as ctx:
            op_ = ctx.enter_context(tc.tile_pool(name="op", bufs=1))
            il = op_.tile([128, NT * 8], dt.int16)
            nc.sync.dma_start(il[:], iol_d[:])
            ih = op_.tile([128, NT * 8], dt.int16)
            nc.sync.dma_start(ih[:], ioh_d[:])
            msk = op_.tile([128, NT], dt.float16)
            nc.sync.dma_start(msk[:], omask_d[:])
            glo = op_.tile([128, NT, 128], dt.float16)
            _gather_split(nc, lambda a, b: glo[:, a:b, :], t1n_lo[:], il, NT,
                          128, 0)
            ghi = op_.tile([128, NT, 128], dt.float16)
            _gather_split(nc, lambda a, b: ghi[:, a:b, :], t1n_hi[:], ih, NT,
                          128, 1)
            mrg = op_.tile([128, NT, 128], dt.float16)
            # mrg = ghi + (glo - ghi) * mask
            nc.vector.tensor_tensor(mrg[:], glo[:], ghi[:], op=OP.subtract)
            nc.vector.tensor_tensor(
                mrg[:], mrg[:],
                msk[:].unsqueeze(2).broadcast_to([128, NT, 128]), op=OP.mult)
            nc.vector.tensor_tensor(mrg[:], mrg[:], ghi[:], op=OP.add)
            nc.sync.dma_start(
                t1n_own[:].rearrange("(t p) c -> p t c", p=128), mrg[:])

        # ------------------------------------------------------------------
        # edge phases
        # ------------------------------------------------------------------
        def edge_phase(ctx, name, tab_lo, tab_hi, n_own, asrep_sb, evict):
            ep = ctx.enter_context(tc.tile_pool(name=name + "e", bufs=3))
            pp = ctx.enter_context(tc.tile_pool(name=name + "p", bufs=2, space="PSUM"))
            for t in range(NT):
                ct = sched.ct[t]
                if ct == 0:
                    continue
                nlo, nhi = sched.n_lo[t], sched.n_hi[t]
                b0 = sched.base[t]
                ixh = ep.tile([128, ct * 8], dt.int16, tag="ixh")
                nc.sync.dma_start(ixh[:], idx_h_d[:, b0 * 8:(b0 + ct) * 8])
                ixn = ep.tile([128, ct * 8], dt.int16, tag="ixn")
                nc.sync.dma_start(ixn[:], idx_nd_d[:, b0 * 8:(b0 + ct) * 8])
                dl = ep.tile([128, ct], dt.float32, tag="dl")
                nc.sync.dma_start(dl[:], dstloc_d[:, b0:b0 + ct])

                # local-table gather + one-hot build first: at the layer-2
                # boundary these depend only on phase-C outputs, so they can
                # proceed while the AllGather (which gates the g gathers
                # below) is still in flight.
                g = ep.tile([128, ct, F], dt.float16, tag="g")
                nd_ = ep.tile([128, ct, 128], dt.float16, tag="nd")
                _gather_split(nc, lambda a, b: nd_[:, a:b, :], n_own, ixn,
                              ct, 128, 1)
                oh = ep.tile([128, ct, 128], dt.float16, tag="oh")
                nc.vector.tensor_tensor(
                    oh[:],
                    iota16[:].unsqueeze(1).broadcast_to([128, ct, 128]),
                    dl[:].unsqueeze(2).broadcast_to([128, ct, 128]),
                    op=OP.is_equal)
                if nlo:
                    _gather_split(nc, lambda a, b: g[:, a:b, :], tab_lo,
                                  ixh, nlo, F, 0)
                if nhi:
                    _gather_split(
                        nc, lambda a, b: g[:, nlo + a:nlo + b, :], tab_hi,
                        ixh[:, nlo * 8:ct * 8], nhi, F, 2)

                # a_src per edge from the gathered rows: ns[e,h] =
                # sum_c g[e,h*C+c] * att_src[h,c]; rhs[:, :, 0:F] is scratch
                # for the product (overwritten later by the fold).
                rhs = ep.tile([128, ct, F + H], dt.float16, tag="rhs")
                nc.vector.tensor_tensor(
                    rhs[:, :, 0:F], g[:],
                    asrep_sb[:].unsqueeze(1).broadcast_to([128, ct, F]),
                    op=OP.mult)
                nsr = ep.tile([128, ct, H], dt.float32, tag="nsr")
                nc.vector.tensor_reduce(
                    nsr[:], rhs[:, :, 0:F].rearrange("p c (h d) -> p c h d",
                                                     h=H),
                    axis=mybir.AxisListType.X, op=OP.add)

                # narrow: alpha = lrelu(a_src + a_dst); ex = exp(alpha)
                alpha = ep.tile([128, ct, H], dt.float32, tag="alpha")
                nc.vector.tensor_tensor(alpha[:], nsr[:], nd_[:, :, H:2 * H],
                                        op=OP.add)
                nc.vector.scalar_tensor_tensor(
                    alpha[:], alpha[:], float(NEG_SLOPE), alpha[:],
                    op0=OP.mult, op1=OP.max)
                nc.scalar.activation(rhs[:, :, F:F + H], alpha[:], ACT.Exp)
                # fold: rhs[:, :, 0:F] = g * ex (per-head broadcast)
                nc.vector.tensor_tensor(
                    rhs[:, :, 0:F].rearrange("p c (h d) -> p c h d", h=H),
                    g[:].rearrange("p c (h d) -> p c h d", h=H),
                    rhs[:, :, F:F + H].unsqueeze(3).broadcast_to(
                        [128, ct, H, F // H]),
                    op=OP.mult)
                # matmul-scatter (one-hot built above, before the g gathers)
                ps = pp.tile([128, F + H], dt.float32, tag="ps")
                for c in range(ct):
                    nc.tensor.matmul(ps[:], oh[:, c, :], rhs[:, c, :],
                                     start=(c == 0), stop=(c == ct - 1))
                evict(ep, pp, t, ps)

        # ---- layer 1 evict: h1 = relu(agg/den + b1); build h1T + h1own ----
        def evict1(ep, pp, t, ps):
            rows = min(128, NSH - t * 128)
            rcp = ep.tile([128, H], dt.float32, tag="rcp")
            nc.vector.reciprocal(rcp[:], ps[:, F:F + H])
            pre = ep.tile([128, F], dt.float32, tag="pre")
            nc.vector.tensor_tensor(
                pre[:].rearrange("p (h d) -> p h d", h=H),
                ps[:, 0:F].rearrange("p (h d) -> p h d", h=H),
                rcp[:].unsqueeze(2).broadcast_to([128, H, F // H]), op=OP.mult)
            nc.vector.tensor_tensor(pre[:], pre[:], b1_sb[:], op=OP.add)
            h1r = ep.tile([128, F], dt.float16, tag="h1r")
            nc.scalar.activation(h1r[:], pre[:], ACT.Relu)
            if TAPS:
                nc.sync.dma_start(tap_h1[t * 128:t * 128 + rows, :], h1r[0:rows, :])
            for b in range(2):
                tp = pp.tile([128, 128], dt.float16, tag="tp")
                nc.tensor.transpose(tp[:], h1r[:, b * 128:(b + 1) * 128], ident16[:])
                nc.scalar.activation(h1T[:, b, t, :], tp[:], ACT.Copy)
            # fused layer-2 dense for this tile: t2h_own rows are complete as
            # soon as the L1 edge phase finishes, so the AllGather can launch
            # without a separate dense pass in between.
            ps2 = pp.tile([128, FA], dt.float32, tag="ps2")
            for b in range(2):
                nc.tensor.matmul(ps2[:], h1T[:, b, t, :], w2a_sb[:, b, :],
                                 start=(b == 0), stop=(b == 1))
            hst = ep.tile([128, F], dt.float16, tag="hst")
            nc.scalar.activation(hst[:], ps2[:, 0:F], ACT.Copy)
            nst = ep.tile([128, 128], dt.float16, tag="nst")
            nc.vector.tensor_copy(nst[:, 0:8], ps2[:, F:FA])
            nc.sync.dma_start(t2h_own[t * 128:t * 128 + rows, :],
                              hst[0:rows, :])
            nc.sync.dma_start(t2n_own[t * 128:(t + 1) * 128, :], nst[:])

        if PH >= 3:
         with ExitStack() as ctx:
            edge_phase(ctx, "l1", t1h_lo[:], t1h_hi[:], t1n_own[:], a1s_sb,
                       evict1)

        # ------------------------------------------------------------------
        # (layer-2 dense is fused into evict1 above)
        # ------------------------------------------------------------------
        # phase D: exchange (h only; per-edge a_src is recomputed from the
        # gathered rows, so no narrow-table exchange is needed)
        # ------------------------------------------------------------------
        if PH >= 5:
         nc.gpsimd.collective_compute(
            "AllGather", OP.bypass, replica_groups=[list(range(NCORES))],
            ins=[t2h_own[:].opt()], outs=[t2h_all[:].opt()])

        # ---- layer 2 evict: h2 = relu(mean_h(agg/den) + b2); FF head ----
        def evict2(ep, pp, t, ps):
            rows = min(128, NSH - t * 128)
            rcp = ep.tile([128, H], dt.float32, tag="rcp")
            nc.vector.reciprocal(rcp[:], ps[:, F:F + H])
            pre = ep.tile([128, H, C2], dt.float32, tag="pre")
            nc.vector.tensor_tensor(
                pre[:], ps[:, 0:F].rearrange("p (h d) -> p h d", h=H),
                rcp[:].unsqueeze(2).broadcast_to([128, H, C2]), op=OP.mult)
            red = ep.tile([128, C2], dt.float32, tag="red")
            nc.vector.tensor_reduce(red[:], pre[:].transpose([0, 2, 1]),
                                    axis=mybir.AxisListType.X, op=OP.add)
            nc.vector.scalar_tensor_tensor(red[:], red[:], 1.0 / H, b2_sb[:],
                                           op0=OP.mult, op1=OP.add)
            h2 = ep.tile([128, 128], dt.float16, tag="h2")
            nc.vector.memset(h2[:, C2:128], 0.0)
            nc.scalar.activation(h2[:, 0:C2], red[:], ACT.Relu)
            if TAPS:
                nc.sync.dma_start(tap_h2[t * 128:t * 128 + rows, :],
                                  h2[0:rows, 0:C2])
            # FF: out = relu(h2 @ ff1 + b1f) @ ff2 + b2f  (square transposes)
            tp = pp.tile([128, 128], dt.float16, tag="tp2", bufs=1)
            nc.tensor.transpose(tp[:], h2[:], ident16[:])
            h2T = ep.tile([C2, 128], dt.float16, tag="h2T")
            nc.scalar.activation(h2T[:], tp[0:C2, :], ACT.Copy)
            pf1 = pp.tile([128, FH], dt.float32, tag="pf1", bufs=1)
            nc.tensor.matmul(pf1[:], h2T[:], ff1_sb[:], start=True, stop=True)
            f1p = ep.tile([128, FH], dt.float32, tag="f1p")
            nc.vector.tensor_tensor(f1p[:], pf1[:], f1b_sb[:], op=OP.add)
            f1 = ep.tile([128, 128], dt.float16, tag="f1")
            nc.vector.memset(f1[:, FH:128], 0.0)
            nc.scalar.activation(f1[:, 0:FH], f1p[:], ACT.Relu)
            if TAPS:
                nc.sync.dma_start(tap_f1[t * 128:t * 128 + rows, :],
                                  f1[0:rows, 0:FH])
            tpf = pp.tile([128, 128], dt.float16, tag="tpf", bufs=1)
            nc.tensor.transpose(tpf[:], f1[:], ident16[:])
            f1T = ep.tile([FH, 128], dt.float16, tag="f1T")
            nc.scalar.activation(f1T[:], tpf[0:FH, :], ACT.Copy)
            pf2 = pp.tile([128, 2], dt.float32, tag="pf2", bufs=1)
            nc.tensor.matmul(pf2[:], f1T[:], ff2_sb[:], start=True, stop=True)
            nc.vector.tensor_tensor(out_stage[:, t, :], pf2[:], f2b_sb[:],
                                    op=OP.add)

        if TAPS:
            A = SPLIT - 128
            nc.sync.dma_start(tap_h[0:128, :], t1h_lo[A:A + 128, :])
            nc.sync.dma_start(tap_h[128:256, :], t1h_hi[0:128, :])
            nc.sync.dma_start(tap_n[0:128, :], t1n_lo[A:A + 128, :])
            nc.sync.dma_start(tap_n[128:256, :], t1n_hi[0:128, :])
            nc.sync.dma_start(tap_own[:], t1n_own[0:256, :])
            nc.sync.dma_start(tap_t2[:], t2h_own[0:256, :])
            nc.sync.dma_start(tap_ag[:], t2h_all[NSH:NSH + 256, :])
        if PH >= 6:
         with ExitStack() as ctx:
            edge_phase(ctx, "l2", t2h_all[0:SPLIT, :], t2h_all[SPLIT:N, :],
                       t2n_own[:], a2s_sb, evict2)

        # final output
        if PH < 6:
            nc.vector.memset(out_stage[:], 0.0)
        full = (NSH // 128) * 128
        if full:
            nc.sync.dma_start(
                out_d[0:full, :].rearrange("(t p) j -> p t j", p=128),
                out_stage[:, 0:full // 128, :])
        if NSH > full:
            nc.sync.dma_start(out_d[full:NSH, :],
                              out_stage[0:NSH - full, NT - 1, :])

    nc.compile()
    return nc


def _wr_rows(nc, dst, r0, rows, st, width, col0=0):
    """DMA staging [128, G, width] (rows r = g*128+p at [p, g]) to DRAM rows
    dst[r0:r0+rows]. col0: starting tile index inside the staging buffer."""
    g_full = rows // 128
    if g_full:
        nc.sync.dma_start(
            dst[r0:r0 + g_full * 128, :].rearrange("(g p) c -> p g c", p=128),
            st[:, col0:col0 + g_full, :])
    rem = rows - g_full * 128
    if rem:
        nc.sync.dma_start(dst[r0 + g_full * 128:r0 + rows, :],
                          st[0:rem, col0 + g_full, :])


# ----------------------------------------------------------------------------
# entry point
# ----------------------------------------------------------------------------

_CACHE = {}
_RUNNER_CACHE = {}


def _make_runner(nc):
    """Persistent jitted shard_map runner for nc (mirrors
    bass2jax.run_bass_via_pjrt but caches the traced computation so repeat
    kernel() calls skip retrace/recompile; inputs are uploaded per call)."""
    import jax
    import concourse.mybir as mybir_
    from concourse.bass2jax import _bass_exec_p, partition_id_tensor, \
        install_neuronx_cc_hook
    from jax.sharding import Mesh, PartitionSpec, NamedSharding
    from jax.experimental.shard_map import shard_map

    install_neuronx_cc_hook()
    partition_name = (nc.partition_id_tensor.name
                      if nc.partition_id_tensor else None)
    in_names, out_names, out_avals, zero_outs = [], [], [], []
    for alloc in nc.m.functions[0].allocations:
        if not isinstance(alloc, mybir_.MemoryLocationSet):
            continue
        name = alloc.memorylocations[0].name
        if alloc.kind == "ExternalInput":
            if name != partition_name:
                in_names.append(name)
        elif alloc.kind == "ExternalOutput":
            shape = tuple(alloc.tensor_shape)
            dtype = mybir_.dt.np(alloc.dtype)
            out_names.append(name)
            out_avals.append(jax.core.ShapedArray(shape, dtype))
            zero_outs.append(np.zeros(shape, dtype))
    n_params = len(in_names)
    n_outs = len(out_avals)
    all_in = list(in_names) + list(out_names)
    if partition_name is not None:
        all_in.append(partition_name)
    donate = tuple(range(n_params, n_params + n_outs))

    def _body(*args):
        operands = list(args)
        if partition_name is not None:
            operands.append(partition_id_tensor())
        return tuple(_bass_exec_p.bind(
            *operands, out_avals=tuple(out_avals), in_names=tuple(all_in),
            out_names=tuple(out_names), lowering_input_output_aliases=(),
            sim_require_finite=True, sim_require_nnan=True, nc=nc))

    devices = jax.devices()[:NCORES]
    mesh = Mesh(np.asarray(devices), ("core",))
    sharded = jax.jit(
        shard_map(_body, mesh=mesh,
                  in_specs=(PartitionSpec("core"),) * (n_params + n_outs),
                  out_specs=(PartitionSpec("core"),) * n_outs,
                  check_rep=False),
        donate_argnums=donate, keep_unused=True)
    sh = NamedSharding(mesh, PartitionSpec("core"))

    def run(in_maps):
        concat_in = [
            jax.device_put(np.concatenate(
                [np.asarray(in_maps[c][n]) for c in range(NCORES)], axis=0),
                sh)
            for n in in_names]
        zs = [jax.device_put(
            np.zeros((NCORES * z.shape[0], *z.shape[1:]), z.dtype), sh)
            for z in zero_outs]
        outs = sharded(*concat_in, *zs)
        return [{name: np.asarray(outs[i]).reshape(
                    NCORES, *out_avals[i].shape)[c]
                 for i, name in enumerate(out_names)}
                for c in range(NCORES)]

    return run


def kernel(x, edge_index, edge_attr, W1, att_src1, att_dst1, b1,
           W2, att_src2, att_dst2, b2, ff1_w, ff1_b, ff2_w, ff2_b):
    x = np.asarray(x, np.float32)
    edge_index = np.asarray(edge_index)
    args = [np.asarray(a, np.float32) for a in
            (W1, att_src1, att_dst1, b1, W2, att_src2, att_dst2, b2,
             ff1_w, ff1_b, ff2_w, ff2_b)]
    in_maps, sched, dims = _prep(x, edge_index, *args)
    key = (dims["N"], dims["IN"], tuple(sched.n_lo), tuple(sched.n_hi))
    if key not in _CACHE:
        nc_built = _build(sched, dims)
        _CACHE[key] = (nc_built, dims["salt_name"])
    nc, salt_name = _CACHE[key]
    salt = np.zeros((1, 4), np.float32)
    for m in in_maps:
        m[salt_name] = salt
    if key not in _RUNNER_CACHE:
        _RUNNER_CACHE[key] = _make_runner(nc)
    res = _RUNNER_CACHE[key](in_maps)
    out = np.concatenate([res[k]["out"] for k in range(NCORES)], axis=0)
    return out.astype(np.float32)



# revision 31
# speedup vs baseline: 1.1570x; 1.0060x over previous
"""GAT (2-layer GATConv + FF head) on 8 Trainium2 NeuronCores.

Strategy (per sharding hint): nodes + incident edges partitioned by
destination across 8 cores; per-edge softmax/scatter local to the
destination shard via one-hot matmul-scatter into PSUM; small weights
replicated. Layer-1 node features are computed fully replicated (input x
is available everywhere); layer-2 features are computed on the owning
shard and exchanged with a single feature AllGather. Random-access reads
(h[src] rows, own-shard a_dst rows) use SWDGE dma_gather from fp16 DRAM
tables. Per-edge a_src is recomputed on the fly from the gathered rows
(dot with att_src on VectorE), so no global narrow tables or narrow
exchange exist. The layer-2 dense is fused into the layer-1 edge-phase
evict so t2h_own completes with the edge phase and the AllGather fires
immediately; each tile's local nd gather + one-hot build are issued
before its table gathers so local work overlaps the collective.

Message path in fp16 (tables, gathered rows, attention weights, matmul
operands); accumulation in fp32 PSUM; per-edge logits in fp32.
"""
import sys
sys.path.insert(0, "/opt/trn_rl_repo")

import numpy as np
from contextlib import ExitStack

import concourse.bass as bass
import concourse.bacc as bacc
import concourse.tile as tile
import concourse.mybir as mybir
from concourse.bass_utils import run_bass_kernel_spmd

dt = mybir.dt
OP = mybir.AluOpType
ACT = mybir.ActivationFunctionType

NCORES = 8
H = 4
NEG_SLOPE = 0.2


# ----------------------------------------------------------------------------
# host-side prep
# ----------------------------------------------------------------------------

def _wrap_idx(idx):
    """Pack an index list into the SWDGE wrapped layout [128, n/16] int16:
    index i -> partition i%16 (replicated to all 8 16-partition groups),
    free offset i//16."""
    n = len(idx)
    assert n % 128 == 0
    out = np.zeros((128, n // 16), np.int16)
    a = np.asarray(idx, np.int16).reshape(n // 16, 16).T  # [16, n/16]
    for r in range(8):
        out[r * 16:(r + 1) * 16, :] = a
    return out


def _pad128(a, fill):
    n = len(a)
    m = ((n + 127) // 128) * 128
    return np.concatenate([a, np.full(m - n, fill, a.dtype)])


class Sched:
    """Static, core-uniform per-tile chunk schedule."""

    def __init__(self, n_lo, n_hi):
        self.n_lo = n_lo          # [NT] chunks for lo-half gathers
        self.n_hi = n_hi          # [NT] chunks for hi-half gathers
        self.ct = [a + b for a, b in zip(n_lo, n_hi)]
        self.base = np.concatenate([[0], np.cumsum(self.ct)]).astype(int)
        self.total = int(self.base[-1])  # total chunks per core


_EDGE_CACHE = {}


def _prep(x, edge_index, W1, att_src1, att_dst1, b1, W2, att_src2, att_dst2,
          b2, ff1_w, ff1_b, ff2_w, ff2_b):
    N, IN = x.shape
    F = W1.shape[1]               # 256
    C1 = F // H
    C2 = W2.shape[1] // H
    NSH = N // NCORES
    NT = (NSH + 127) // 128
    NLO_T = (N // 2) // 128
    SPLIT = NLO_T * 128

    import hashlib as _hl
    ekey = (N, _hl.sha1(np.ascontiguousarray(edge_index).tobytes())
            .hexdigest())
    cached = _EDGE_CACHE.get(ekey)
    if cached is not None:
        (sched, idx_h, idx_nd, dstloc, idx_own_lo, idx_own_hi,
         own_mask) = cached
        return _prep_finish(x, W1, att_src1, att_dst1, b1, W2, att_src2,
                            att_dst2, b2, ff1_w, ff1_b, ff2_w, ff2_b,
                            sched, idx_h, idx_nd, dstloc, idx_own_lo,
                            idx_own_hi, own_mask,
                            N, IN, F, NSH, NT, SPLIT, C1, C2)

    E = edge_index.shape[1]
    ar = np.arange(N, dtype=np.int64)
    src = np.concatenate([edge_index[0], ar])
    dst = np.concatenate([edge_index[1], ar])

    shard = dst // NSH
    dstloc_all = dst - shard * NSH

    # group edges per (core, tile, half)
    per = [[[None, None] for _ in range(NT)] for _ in range(NCORES)]
    for k in range(NCORES):
        m = shard == k
        s_k, dl_k = src[m], dstloc_all[m]
        t_k = dl_k // 128
        for t in range(NT):
            mt = t_k == t
            s_t, dl_t = s_k[mt], dl_k[mt]
            lo = s_t < SPLIT
            per[k][t][0] = (s_t[lo], dl_t[lo])
            per[k][t][1] = (s_t[~lo] - SPLIT, dl_t[~lo])

    n_lo = [max((len(per[k][t][0][0]) + 127) // 128 for k in range(NCORES))
            for t in range(NT)]
    n_hi = [max((len(per[k][t][1][0]) + 127) // 128 for k in range(NCORES))
            for t in range(NT)]
    sched = Sched(n_lo, n_hi)

    # per-core edge arrays in schedule order
    idx_h = []       # [128, total*8] int16  (gather idx, lo/hi-local rows)
    idx_nd = []      # [128, total*8] int16  (dstlocal shard rows)
    dstloc = []      # [128, total] f32      (tile-local dst or -1)
    for k in range(NCORES):
        ih = np.zeros((128, sched.total * 8), np.int16)
        nd = np.zeros((128, sched.total * 8), np.int16)
        dl = np.full((128, sched.total), -1.0, np.float32)
        for t in range(NT):
            off = sched.base[t]
            for half, nch in ((0, n_lo[t]), (1, n_hi[t])):
                if nch == 0:
                    continue
                s_t, dl_t = per[k][t][half]
                ne = nch * 128
                sp = _pad128(np.concatenate([s_t, np.zeros(ne - len(s_t),
                                                           np.int64)]), 0)[:ne]
                sp[len(s_t):] = 0
                dlp = np.full(ne, -1.0, np.float32)
                dlp[:len(dl_t)] = (dl_t - t * 128).astype(np.float32)
                ndp = np.zeros(ne, np.int64)
                ndp[:len(dl_t)] = dl_t
                ih[:, off * 8:(off + nch) * 8] = _wrap_idx(sp)
                nd[:, off * 8:(off + nch) * 8] = _wrap_idx(ndp)
                dl[:, off:off + nch] = dlp.reshape(nch, 128).T
                off += nch
        idx_h.append(ih)
        idx_nd.append(nd)
        dstloc.append(dl)

    # own-narrow build: gather T1n rows for own shard (lo/hi + select mask)
    own_rows_pad = NT * 128
    idx_own_lo, idx_own_hi, own_mask = [], [], []
    for k in range(NCORES):
        rows = np.arange(k * NSH, (k + 1) * NSH)
        rows = np.concatenate([rows, np.full(own_rows_pad - NSH, rows[0])])
        is_lo = rows < SPLIT
        lo_i = np.where(is_lo, rows, 0)
        hi_i = np.where(is_lo, 0, rows - SPLIT)
        idx_own_lo.append(_wrap_idx(lo_i))
        idx_own_hi.append(_wrap_idx(hi_i))
        m = np.zeros((128, NT), np.float16)
        m[:, :] = is_lo.reshape(NT, 128).T.astype(np.float16)
        own_mask.append(m)

    _EDGE_CACHE[ekey] = (sched, idx_h, idx_nd, dstloc, idx_own_lo,
                         idx_own_hi, own_mask)
    return _prep_finish(x, W1, att_src1, att_dst1, b1, W2, att_src2,
                        att_dst2, b2, ff1_w, ff1_b, ff2_w, ff2_b,
                        sched, idx_h, idx_nd, dstloc, idx_own_lo,
                        idx_own_hi, own_mask,
                        N, IN, F, NSH, NT, SPLIT, C1, C2)


def _prep_finish(x, W1, att_src1, att_dst1, b1, W2, att_src2, att_dst2, b2,
                 ff1_w, ff1_b, ff2_w, ff2_b, sched, idx_h, idx_nd, dstloc,
                 idx_own_lo, idx_own_hi, own_mask,
                 N, IN, F, NSH, NT, SPLIT, C1, C2):
    # weights
    def aug(W, a_s, a_d, C):
        v_s = np.einsum("fhc,hc->fh", W.reshape(-1, H, C), a_s)
        v_d = np.einsum("fhc,hc->fh", W.reshape(-1, H, C), a_d)
        return np.concatenate([W, v_s, v_d], axis=1).astype(np.float16)

    W1aug = aug(W1, att_src1, att_dst1, C1)              # [IN, F+8]
    W2aug = aug(W2, att_src2, att_dst2, C2)              # [F, F+8]
    W2aug_pk = W2aug.reshape(2, 128, F + 8).transpose(1, 0, 2).copy()

    xT16 = np.ascontiguousarray(x.T).astype(np.float16)  # [IN, N]

    iota16 = np.tile(np.arange(128, dtype=np.float16), (128, 1))
    ident16 = np.eye(128, dtype=np.float16)

    const = {
        "xT16": xT16, "W1aug": W1aug, "W2aug": W2aug_pk,
        "iota16": iota16, "ident16": ident16,
        "a1srep": np.tile(att_src1.astype(np.float16).reshape(1, F),
                          (128, 1)),
        "a2srep": np.tile(att_src2.astype(np.float16).reshape(1, F),
                          (128, 1)),
        "b1rep": np.tile(b1.astype(np.float32), (128, 1)),
        "b2rep": np.tile(b2.astype(np.float32), (128, 1)),
        "f1brep": np.tile(ff1_b.astype(np.float32), (128, 1)),
        "f2brep": np.tile(ff2_b.astype(np.float32), (128, 1)),
        "ff1w16": ff1_w.astype(np.float16),
        "ff2w16": ff2_w.astype(np.float16),
    }

    in_maps = []
    for k in range(NCORES):
        m = dict(const)
        m.update({
            "idx_h": idx_h[k], "idx_nd": idx_nd[k], "dstloc": dstloc[k],
            "idx_own_lo": idx_own_lo[k], "idx_own_hi": idx_own_hi[k],
            "own_mask": own_mask[k],
        })
        in_maps.append(m)

    dims = dict(N=N, IN=IN, F=F, NSH=NSH, NT=NT, SPLIT=SPLIT,
                NLO=SPLIT, NHI=N - SPLIT, C2=C2, FH=ff1_w.shape[1])
    return in_maps, sched, dims


# ----------------------------------------------------------------------------
# device program
# ----------------------------------------------------------------------------

def _gather_split(nc, out_ap_fn, tab, idx_sb, n_chunks, elem, q0):
    """Emit dma_gather calls capped at 8 chunks (1024 idxs) each.
    out_ap_fn(c0, c1) -> output AP for chunk range; idx_sb indexed per chunk."""
    c0 = 0
    q = q0
    while c0 < n_chunks:
        c1 = min(c0 + 8, n_chunks)
        nc.gpsimd.dma_gather(
            out_ap_fn(c0, c1), tab, idx_sb[:, c0 * 8:c1 * 8],
            num_idxs=(c1 - c0) * 128, num_idxs_reg=(c1 - c0) * 128,
            elem_size=elem, queue_num=q % 4)
        q += 1
        c0 = c1


def _build(sched, dims):
    import os, hashlib
    PH = int(os.environ.get('K_PHASES', '6'))
    TAPS = int(os.environ.get('K_TAPS', '0'))
    N, IN, F, NSH, NT = dims["N"], dims["IN"], dims["F"], dims["NSH"], dims["NT"]
    NLO, NHI, SPLIT = dims["NLO"], dims["NHI"], dims["SPLIT"]
    C2, FH = dims["C2"], dims["FH"]
    FA = F + 8
    NTG = (N + 127) // 128          # global node tiles
    NTG_LO = NLO // 128

    nc = bacc.Bacc("TRN2", target_bir_lowering=False, num_devices=NCORES,
                   num_swdge_queues=4)
    # The neuronx compile cache keys on the jit signature only (the embedded
    # BIR is not hashed), so two different programs with identical I/O would
    # alias to one NEFF. A content-named dummy input de-aliases them.
    with open(__file__, "rb") as _f:
        _salt = hashlib.sha256(
            _f.read() + repr((sched.n_lo, sched.n_hi, sorted(dims.items()),
                              PH)).encode()).hexdigest()[:16]
    nc.dram_tensor(f"salt_{_salt}", [1, 4], dt.float32, kind="ExternalInput")
    dims["salt_name"] = f"salt_{_salt}"

    # inputs
    xT16 = nc.dram_tensor("xT16", [IN, N], dt.float16, kind="ExternalInput")
    W1aug = nc.dram_tensor("W1aug", [IN, FA], dt.float16, kind="ExternalInput")
    W2aug = nc.dram_tensor("W2aug", [128, 2, FA], dt.float16, kind="ExternalInput")
    iota_in = nc.dram_tensor("iota16", [128, 128], dt.float16, kind="ExternalInput")
    ident_in = nc.dram_tensor("ident16", [128, 128], dt.float16, kind="ExternalInput")
    b1rep = nc.dram_tensor("b1rep", [128, F], dt.float32, kind="ExternalInput")
    b2rep = nc.dram_tensor("b2rep", [128, C2], dt.float32, kind="ExternalInput")
    f1brep = nc.dram_tensor("f1brep", [128, FH], dt.float32, kind="ExternalInput")
    f2brep = nc.dram_tensor("f2brep", [128, 2], dt.float32, kind="ExternalInput")
    ff1w16 = nc.dram_tensor("ff1w16", [C2, FH], dt.float16, kind="ExternalInput")
    ff2w16 = nc.dram_tensor("ff2w16", [FH, 2], dt.float16, kind="ExternalInput")
    a1srep_d = nc.dram_tensor("a1srep", [128, F], dt.float16, kind="ExternalInput")
    a2srep_d = nc.dram_tensor("a2srep", [128, F], dt.float16, kind="ExternalInput")
    idx_h_d = nc.dram_tensor("idx_h", [128, sched.total * 8], dt.int16, kind="ExternalInput")
    idx_nd_d = nc.dram_tensor("idx_nd", [128, sched.total * 8], dt.int16, kind="ExternalInput")
    dstloc_d = nc.dram_tensor("dstloc", [128, sched.total], dt.float32, kind="ExternalInput")
    iol_d = nc.dram_tensor("idx_own_lo", [128, NT * 8], dt.int16, kind="ExternalInput")
    ioh_d = nc.dram_tensor("idx_own_hi", [128, NT * 8], dt.int16, kind="ExternalInput")
    omask_d = nc.dram_tensor("own_mask", [128, NT], dt.float16, kind="ExternalInput")

    out_d = nc.dram_tensor("out", [NSH, 2], dt.float32, kind="ExternalOutput")
    if TAPS:
        tap_h = nc.dram_tensor("tap_h", [256, F], dt.float16, kind="ExternalOutput")
        tap_n = nc.dram_tensor("tap_n", [256, 128], dt.float16, kind="ExternalOutput")
        tap_own = nc.dram_tensor("tap_own", [256, 128], dt.float16, kind="ExternalOutput")
        tap_h1 = nc.dram_tensor("tap_h1", [NSH, F], dt.float16, kind="ExternalOutput")
        tap_t2 = nc.dram_tensor("tap_t2", [256, F], dt.float16, kind="ExternalOutput")
        tap_ag = nc.dram_tensor("tap_ag", [256, F], dt.float16, kind="ExternalOutput")
        tap_h2 = nc.dram_tensor("tap_h2", [NSH, C2], dt.float16, kind="ExternalOutput")
        tap_f1 = nc.dram_tensor("tap_f1", [NSH, FH], dt.float16, kind="ExternalOutput")

    with tile.TileContext(nc) as tc, ExitStack() as octx:
        # persistent pools
        dram = octx.enter_context(tc.tile_pool(name="dram", bufs=1, space="DRAM"))
        cpool = octx.enter_context(tc.tile_pool(name="const", bufs=1))
        stash = octx.enter_context(tc.tile_pool(name="stash", bufs=1))

        # DRAM tables
        t1h_lo = dram.tile([NLO, F], dt.float16)
        t1h_hi = dram.tile([NHI, F], dt.float16)
        t1n_lo = dram.tile([NLO, 128], dt.float16)
        t1n_hi = dram.tile([NHI, 128], dt.float16)
        t1n_own = dram.tile([NT * 128, 128], dt.float16)
        t2h_own = dram.tile([NSH, F], dt.float16)
        t2n_own = dram.tile([NT * 128, 128], dt.float16)
        t2h_all = dram.tile([N, F], dt.float16, addr_space="Shared")

        # constants in SBUF
        iota16 = cpool.tile([128, 128], dt.float16)
        nc.sync.dma_start(iota16[:], iota_in[:])
        ident16 = cpool.tile([128, 128], dt.float16)
        nc.sync.dma_start(ident16[:], ident_in[:])
        w1a_sb = cpool.tile([IN, FA], dt.float16)
        nc.sync.dma_start(w1a_sb[:], W1aug[:])
        w2a_sb = cpool.tile([128, 2, FA], dt.float16)
        nc.sync.dma_start(w2a_sb[:], W2aug[:])
        b1_sb = cpool.tile([128, F], dt.float32)
        nc.sync.dma_start(b1_sb[:], b1rep[:])
        b2_sb = cpool.tile([128, C2], dt.float32)
        nc.sync.dma_start(b2_sb[:], b2rep[:])
        f1b_sb = cpool.tile([128, FH], dt.float32)
        nc.sync.dma_start(f1b_sb[:], f1brep[:])
        f2b_sb = cpool.tile([128, 2], dt.float32)
        nc.sync.dma_start(f2b_sb[:], f2brep[:])
        ff1_sb = cpool.tile([C2, FH], dt.float16)
        nc.sync.dma_start(ff1_sb[:], ff1w16[:])
        ff2_sb = cpool.tile([FH, 2], dt.float16)
        nc.sync.dma_start(ff2_sb[:], ff2w16[:])
        a1s_sb = cpool.tile([128, F], dt.float16)
        nc.sync.dma_start(a1s_sb[:], a1srep_d[:])
        a2s_sb = cpool.tile([128, F], dt.float16)
        nc.sync.dma_start(a2s_sb[:], a2srep_d[:])

        # layer-1 hidden transposed, kept in SBUF for the layer-2 dense
        h1T = stash.tile([128, 2, NT, 128], dt.float16)
        out_stage = stash.tile([128, NT, 2], dt.float32)

        # ------------------------------------------------------------------
        # phase A: full replicated layer-1 dense -> T1 tables
        # ------------------------------------------------------------------
        with ExitStack() as ctx:
            xp = ctx.enter_context(tc.tile_pool(name="xp", bufs=2))
            pp = ctx.enter_context(tc.tile_pool(name="pp", bufs=4, space="PSUM"))
            sp = ctx.enter_context(tc.tile_pool(name="sp", bufs=2))

            G = 8
            groups = []
            m0 = 0
            while m0 < NTG:
                g = min(G, NTG - m0)
                groups.append((m0, g))
                m0 += g
            for (m0, g) in groups:
                xs = xp.tile([IN, G * 128], dt.float16, tag="xs")
                rows_t = min(g * 128, N - m0 * 128)
                nc.sync.dma_start(xs[:, 0:rows_t], xT16[:, m0 * 128:m0 * 128 + rows_t])
                hst = sp.tile([128, G, F], dt.float16, tag="hst")
                nst = sp.tile([128, G, 128], dt.float16, tag="nst")
                for j in range(g):
                    m = m0 + j
                    rows = min(128, N - m * 128)
                    ps = pp.tile([128, FA], dt.float32, tag="ps")
                    nc.tensor.matmul(ps[0:rows, :], xs[:, j * 128:j * 128 + rows],
                                     w1a_sb[:], start=True, stop=True)
                    nc.scalar.activation(hst[0:rows, j, :], ps[0:rows, 0:F], ACT.Copy)
                    nc.vector.tensor_copy(nst[0:rows, j, 0:8], ps[0:rows, F:FA])
                # group DMA to tables (handle a group straddling SPLIT)
                r0 = m0 * 128
                rows_t = min(g * 128, N - r0)
                if r0 + rows_t <= SPLIT:
                    dst_h, dst_n, rr = t1h_lo, t1n_lo, r0
                elif r0 >= SPLIT:
                    dst_h, dst_n, rr = t1h_hi, t1n_hi, r0 - SPLIT
                else:
                    dst_h = None
                if dst_h is not None:
                    _wr_rows(nc, dst_h, rr, rows_t, hst, F)
                    _wr_rows(nc, dst_n, rr, rows_t, nst, 128)
                else:
                    a = SPLIT - r0
                    _wr_rows(nc, t1h_lo, r0, a, hst, F)
                    _wr_rows(nc, t1n_lo, r0, a, nst, 128)
                    _wr_rows(nc, t1h_hi, 0, rows_t - a, hst, F, col0=a // 128)
                    _wr_rows(nc, t1n_hi, 0, rows_t - a, nst, 128, col0=a // 128)

        # ------------------------------------------------------------------
        # own-narrow table for layer 1 (gather own rows from t1n lo/hi)
        # ------------------------------------------------------------------
        if PH >= 2:
         with ExitStack() as ctx:
            op_ = ctx.enter_context(tc.tile_pool(name="op", bufs=1))
            il = op_.tile([128, NT * 8], dt.int16)
            nc.sync.dma_start(il[:], iol_d[:])
            ih = op_.tile([128, NT * 8], dt.int16)
            nc.sync.dma_start(ih[:], ioh_d[:])
            msk = op_.tile([128, NT], dt.float16)
            nc.sync.dma_start(msk[:], omask_d[:])
            glo = op_.tile([128, NT, 128], dt.float16)
            _gather_split(nc, lambda a, b: glo[:, a:b, :], t1n_lo[:], il, NT,
                          128, 0)
            ghi = op_.tile([128, NT, 128], dt.float16)
            _gather_split(nc, lambda a, b: ghi[:, a:b, :], t1n_hi[:], ih, NT,
                          128, 1)
            mrg = op_.tile([128, NT, 128], dt.float16)
            # mrg = ghi + (glo - ghi) * mask
            nc.vector.tensor_tensor(mrg[:], glo[:], ghi[:], op=OP.subtract)
            nc.vector.tensor_tensor(
                mrg[:], mrg[:],
                msk[:].unsqueeze(2).broadcast_to([128, NT, 128]), op=OP.mult)
            nc.vector.tensor_tensor(mrg[:], mrg[:], ghi[:], op=OP.add)
            nc.sync.dma_start(
                t1n_own[:].rearrange("(t p) c -> p t c", p=128), mrg[:])

        # ------------------------------------------------------------------
        # edge phases
        # ------------------------------------------------------------------
        def edge_phase(ctx, name, tab_lo, tab_hi, n_own, asrep_sb, evict):
            ep = ctx.enter_context(tc.tile_pool(name=name + "e", bufs=3))
            pp = ctx.enter_context(tc.tile_pool(name=name + "p", bufs=2, space="PSUM"))
            for t in range(NT):
                ct = sched.ct[t]
                if ct == 0:
                    continue
                nlo, nhi = sched.n_lo[t], sched.n_hi[t]
                b0 = sched.base[t]
                ixh = ep.tile([128, ct * 8], dt.int16, tag="ixh")
                nc.sync.dma_start(ixh[:], idx_h_d[:, b0 * 8:(b0 + ct) * 8])
                ixn = ep.tile([128, ct * 8], dt.int16, tag="ixn")
                nc.sync.dma_start(ixn[:], idx_nd_d[:, b0 * 8:(b0 + ct) * 8])
                dl = ep.tile([128, ct], dt.float32, tag="dl")
                nc.sync.dma_start(dl[:], dstloc_d[:, b0:b0 + ct])

                # local-table gather + one-hot build first: at the layer-2
                # boundary these depend only on phase-C outputs, so they can
                # proceed while the AllGather (which gates the g gathers
                # below) is still in flight.
                g = ep.tile([128, ct, F], dt.float16, tag="g")
                nd_ = ep.tile([128, ct, 128], dt.float16, tag="nd")
                _gather_split(nc, lambda a, b: nd_[:, a:b, :], n_own, ixn,
                              ct, 128, 1)
                oh = ep.tile([128, ct, 128], dt.float16, tag="oh")
                nc.vector.tensor_tensor(
                    oh[:],
                    iota16[:].unsqueeze(1).broadcast_to([128, ct, 128]),
                    dl[:].unsqueeze(2).broadcast_to([128, ct, 128]),
                    op=OP.is_equal)
                if nlo:
                    _gather_split(nc, lambda a, b: g[:, a:b, :], tab_lo,
                                  ixh, nlo, F, 0)
                if nhi:
                    _gather_split(
                        nc, lambda a, b: g[:, nlo + a:nlo + b, :], tab_hi,
                        ixh[:, nlo * 8:ct * 8], nhi, F, 2)

                # a_src per edge from the gathered rows: ns[e,h] =
                # sum_c g[e,h*C+c] * att_src[h,c]; rhs[:, :, 0:F] is scratch
                # for the product (overwritten later by the fold).
                rhs = ep.tile([128, ct, F + H], dt.float16, tag="rhs")
                nc.vector.tensor_tensor(
                    rhs[:, :, 0:F], g[:],
                    asrep_sb[:].unsqueeze(1).broadcast_to([128, ct, F]),
                    op=OP.mult)
                nsr = ep.tile([128, ct, H], dt.float32, tag="nsr")
                nc.vector.tensor_reduce(
                    nsr[:], rhs[:, :, 0:F].rearrange("p c (h d) -> p c h d",
                                                     h=H),
                    axis=mybir.AxisListType.X, op=OP.add)

                # narrow: alpha = lrelu(a_src + a_dst); ex = exp(alpha)
                alpha = ep.tile([128, ct, H], dt.float32, tag="alpha")
                nc.vector.tensor_tensor(alpha[:], nsr[:], nd_[:, :, H:2 * H],
                                        op=OP.add)
                nc.vector.scalar_tensor_tensor(
                    alpha[:], alpha[:], float(NEG_SLOPE), alpha[:],
                    op0=OP.mult, op1=OP.max)
                nc.scalar.activation(rhs[:, :, F:F + H], alpha[:], ACT.Exp)
                # fold: rhs[:, :, 0:F] = g * ex (per-head broadcast)
                nc.vector.tensor_tensor(
                    rhs[:, :, 0:F].rearrange("p c (h d) -> p c h d", h=H),
                    g[:].rearrange("p c (h d) -> p c h d", h=H),
                    rhs[:, :, F:F + H].unsqueeze(3).broadcast_to(
                        [128, ct, H, F // H]),
                    op=OP.mult)
                # matmul-scatter (one-hot built above, before the g gathers)
                ps = pp.tile([128, F + H], dt.float32, tag="ps", bufs=3)
                for c in range(ct):
                    nc.tensor.matmul(ps[:], oh[:, c, :], rhs[:, c, :],
                                     start=(c == 0), stop=(c == ct - 1))
                evict(ep, pp, t, ps)

        # ---- layer 1 evict: h1 = relu(agg/den + b1); build h1T + h1own ----
        def evict1(ep, pp, t, ps):
            rows = min(128, NSH - t * 128)
            rcp = ep.tile([128, H], dt.float32, tag="rcp")
            nc.vector.reciprocal(rcp[:], ps[:, F:F + H])
            pre = ep.tile([128, F], dt.float32, tag="pre")
            nc.vector.tensor_tensor(
                pre[:].rearrange("p (h d) -> p h d", h=H),
                ps[:, 0:F].rearrange("p (h d) -> p h d", h=H),
                rcp[:].unsqueeze(2).broadcast_to([128, H, F // H]), op=OP.mult)
            nc.vector.tensor_tensor(pre[:], pre[:], b1_sb[:], op=OP.add)
            h1r = ep.tile([128, F], dt.float16, tag="h1r")
            nc.scalar.activation(h1r[:], pre[:], ACT.Relu)
            if TAPS:
                nc.sync.dma_start(tap_h1[t * 128:t * 128 + rows, :], h1r[0:rows, :])
            for b in range(2):
                tp = pp.tile([128, 128], dt.float16, tag="tp")
                nc.tensor.transpose(tp[:], h1r[:, b * 128:(b + 1) * 128], ident16[:])
                nc.scalar.activation(h1T[:, b, t, :], tp[:], ACT.Copy)
            # fused layer-2 dense for this tile: t2h_own rows are complete as
            # soon as the L1 edge phase finishes, so the AllGather can launch
            # without a separate dense pass in between.
            ps2 = pp.tile([128, FA], dt.float32, tag="ps2")
            for b in range(2):
                nc.tensor.matmul(ps2[:], h1T[:, b, t, :], w2a_sb[:, b, :],
                                 start=(b == 0), stop=(b == 1))
            hst = ep.tile([128, F], dt.float16, tag="hst")
            nc.scalar.activation(hst[:], ps2[:, 0:F], ACT.Copy)
            nst = ep.tile([128, 128], dt.float16, tag="nst")
            nc.vector.tensor_copy(nst[:, 0:8], ps2[:, F:FA])
            nc.sync.dma_start(t2h_own[t * 128:t * 128 + rows, :],
                              hst[0:rows, :])
            nc.sync.dma_start(t2n_own[t * 128:(t + 1) * 128, :], nst[:])

        if PH >= 3:
         with ExitStack() as ctx:
            edge_phase(ctx, "l1", t1h_lo[:], t1h_hi[:], t1n_own[:], a1s_sb,
                       evict1)

        # ------------------------------------------------------------------
        # (layer-2 dense is fused into evict1 above)
        # ------------------------------------------------------------------
        # phase D: exchange (h only; per-edge a_src is recomputed from the
        # gathered rows, so no narrow-table exchange is needed)
        # ------------------------------------------------------------------
        if PH >= 5:
         nc.gpsimd.collective_compute(
            "AllGather", OP.bypass, replica_groups=[list(range(NCORES))],
            ins=[t2h_own[:].opt()], outs=[t2h_all[:].opt()])

        # ---- layer 2 evict: h2 = relu(mean_h(agg/den) + b2); FF head ----
        def evict2(ep, pp, t, ps):
            rows = min(128, NSH - t * 128)
            rcp = ep.tile([128, H], dt.float32, tag="rcp")
            nc.vector.reciprocal(rcp[:], ps[:, F:F + H])
            pre = ep.tile([128, H, C2], dt.float32, tag="pre")
            nc.vector.tensor_tensor(
                pre[:], ps[:, 0:F].rearrange("p (h d) -> p h d", h=H),
                rcp[:].unsqueeze(2).broadcast_to([128, H, C2]), op=OP.mult)
            red = ep.tile([128, C2], dt.float32, tag="red")
            nc.vector.tensor_reduce(red[:], pre[:].transpose([0, 2, 1]),
                                    axis=mybir.AxisListType.X, op=OP.add)
            nc.vector.scalar_tensor_tensor(red[:], red[:], 1.0 / H, b2_sb[:],
                                           op0=OP.mult, op1=OP.add)
            h2 = ep.tile([128, 128], dt.float16, tag="h2")
            nc.vector.memset(h2[:, C2:128], 0.0)
            nc.scalar.activation(h2[:, 0:C2], red[:], ACT.Relu)
            if TAPS:
                nc.sync.dma_start(tap_h2[t * 128:t * 128 + rows, :],
                                  h2[0:rows, 0:C2])
            # FF: out = relu(h2 @ ff1 + b1f) @ ff2 + b2f  (square transposes)
            tp = pp.tile([128, 128], dt.float16, tag="tp2", bufs=1)
            nc.tensor.transpose(tp[:], h2[:], ident16[:])
            h2T = ep.tile([C2, 128], dt.float16, tag="h2T")
            nc.scalar.activation(h2T[:], tp[0:C2, :], ACT.Copy)
            pf1 = pp.tile([128, FH], dt.float32, tag="pf1", bufs=1)
            nc.tensor.matmul(pf1[:], h2T[:], ff1_sb[:], start=True, stop=True)
            f1p = ep.tile([128, FH], dt.float32, tag="f1p")
            nc.vector.tensor_tensor(f1p[:], pf1[:], f1b_sb[:], op=OP.add)
            f1 = ep.tile([128, 128], dt.float16, tag="f1")
            nc.vector.memset(f1[:, FH:128], 0.0)
            nc.scalar.activation(f1[:, 0:FH], f1p[:], ACT.Relu)
            if TAPS:
                nc.sync.dma_start(tap_f1[t * 128:t * 128 + rows, :],
                                  f1[0:rows, 0:FH])
            tpf = pp.tile([128, 128], dt.float16, tag="tpf", bufs=1)
            nc.tensor.transpose(tpf[:], f1[:], ident16[:])
            f1T = ep.tile([FH, 128], dt.float16, tag="f1T")
            nc.scalar.activation(f1T[:], tpf[0:FH, :], ACT.Copy)
            pf2 = pp.tile([128, 2], dt.float32, tag="pf2", bufs=1)
            nc.tensor.matmul(pf2[:], f1T[:], ff2_sb[:], start=True, stop=True)
            nc.vector.tensor_tensor(out_stage[:, t, :], pf2[:], f2b_sb[:],
                                    op=OP.add)

        if TAPS:
            A = SPLIT - 128
            nc.sync.dma_start(tap_h[0:128, :], t1h_lo[A:A + 128, :])
            nc.sync.dma_start(tap_h[128:256, :], t1h_hi[0:128, :])
            nc.sync.dma_start(tap_n[0:128, :], t1n_lo[A:A + 128, :])
            nc.sync.dma_start(tap_n[128:256, :], t1n_hi[0:128, :])
            nc.sync.dma_start(tap_own[:], t1n_own[0:256, :])
            nc.sync.dma_start(tap_t2[:], t2h_own[0:256, :])
            nc.sync.dma_start(tap_ag[:], t2h_all[NSH:NSH + 256, :])
        if PH >= 6:
         with ExitStack() as ctx:
            edge_phase(ctx, "l2", t2h_all[0:SPLIT, :], t2h_all[SPLIT:N, :],
                       t2n_own[:], a2s_sb, evict2)

        # final output
        if PH < 6:
            nc.vector.memset(out_stage[:], 0.0)
        full = (NSH // 128) * 128
        if full:
            nc.sync.dma_start(
                out_d[0:full, :].rearrange("(t p) j -> p t j", p=128),
                out_stage[:, 0:full // 128, :])
        if NSH > full:
            nc.sync.dma_start(out_d[full:NSH, :],
                              out_stage[0:NSH - full, NT - 1, :])

    nc.compile()
    return nc


def _wr_rows(nc, dst, r0, rows, st, width, col0=0):
    """DMA staging [128, G, width] (rows r = g*128+p at [p, g]) to DRAM rows
    dst[r0:r0+rows]. col0: starting tile index inside the staging buffer."""
    g_full = rows // 128
    if g_full:
        nc.sync.dma_start(
            dst[r0:r0 + g_full * 128, :].rearrange("(g p) c -> p g c", p=128),
            st[:, col0:col0 + g_full, :])
    rem = rows - g_full * 128
    if rem:
        nc.sync.dma_start(dst[r0 + g_full * 128:r0 + rows, :],
                          st[0:rem, col0 + g_full, :])


# ----------------------------------------------------------------------------
# entry point
# ----------------------------------------------------------------------------

_CACHE = {}
_RUNNER_CACHE = {}


def _make_runner(nc):
    """Persistent jitted shard_map runner for nc (mirrors
    bass2jax.run_bass_via_pjrt but caches the traced computation so repeat
    kernel() calls skip retrace/recompile; inputs are uploaded per call)."""
    import jax
    import concourse.mybir as mybir_
    from concourse.bass2jax import _bass_exec_p, partition_id_tensor, \
        install_neuronx_cc_hook
    from jax.sharding import Mesh, PartitionSpec, NamedSharding
    from jax.experimental.shard_map import shard_map

    install_neuronx_cc_hook()
    partition_name = (nc.partition_id_tensor.name
                      if nc.partition_id_tensor else None)
    in_names, out_names, out_avals, zero_outs = [], [], [], []
    for alloc in nc.m.functions[0].allocations:
        if not isinstance(alloc, mybir_.MemoryLocationSet):
            continue
        name = alloc.memorylocations[0].name
        if alloc.kind == "ExternalInput":
            if name != partition_name:
                in_names.append(name)
        elif alloc.kind == "ExternalOutput":
            shape = tuple(alloc.tensor_shape)
            dtype = mybir_.dt.np(alloc.dtype)
            out_names.append(name)
            out_avals.append(jax.core.ShapedArray(shape, dtype))
            zero_outs.append(np.zeros(shape, dtype))
    n_params = len(in_names)
    n_outs = len(out_avals)
    all_in = list(in_names) + list(out_names)
    if partition_name is not None:
        all_in.append(partition_name)
    donate = tuple(range(n_params, n_params + n_outs))

    def _body(*args):
        operands = list(args)
        if partition_name is not None:
            operands.append(partition_id_tensor())
        return tuple(_bass_exec_p.bind(
            *operands, out_avals=tuple(out_avals), in_names=tuple(all_in),
            out_names=tuple(out_names), lowering_input_output_aliases=(),
            sim_require_finite=True, sim_require_nnan=True, nc=nc))

    devices = jax.devices()[:NCORES]
    mesh = Mesh(np.asarray(devices), ("core",))
    sharded = jax.jit(
        shard_map(_body, mesh=mesh,
                  in_specs=(PartitionSpec("core"),) * (n_params + n_outs),
                  out_specs=(PartitionSpec("core"),) * n_outs,
                  check_rep=False),
        donate_argnums=donate, keep_unused=True)
    sh = NamedSharding(mesh, PartitionSpec("core"))

    def run(in_maps):
        concat_in = [
            jax.device_put(np.concatenate(
                [np.asarray(in_maps[c][n]) for c in range(NCORES)], axis=0),
                sh)
            for n in in_names]
        zs = [jax.device_put(
            np.zeros((NCORES * z.shape[0], *z.shape[1:]), z.dtype), sh)
            for z in zero_outs]
        outs = sharded(*concat_in, *zs)
        return [{name: np.asarray(outs[i]).reshape(
                    NCORES, *out_avals[i].shape)[c]
                 for i, name in enumerate(out_names)}
                for c in range(NCORES)]

    return run


def kernel(x, edge_index, edge_attr, W1, att_src1, att_dst1, b1,
           W2, att_src2, att_dst2, b2, ff1_w, ff1_b, ff2_w, ff2_b):
    x = np.asarray(x, np.float32)
    edge_index = np.asarray(edge_index)
    args = [np.asarray(a, np.float32) for a in
            (W1, att_src1, att_dst1, b1, W2, att_src2, att_dst2, b2,
             ff1_w, ff1_b, ff2_w, ff2_b)]
    in_maps, sched, dims = _prep(x, edge_index, *args)
    key = (dims["N"], dims["IN"], tuple(sched.n_lo), tuple(sched.n_hi))
    if key not in _CACHE:
        nc_built = _build(sched, dims)
        _CACHE[key] = (nc_built, dims["salt_name"])
    nc, salt_name = _CACHE[key]
    salt = np.zeros((1, 4), np.float32)
    for m in in_maps:
        m[salt_name] = salt
    if key not in _RUNNER_CACHE:
        _RUNNER_CACHE[key] = _make_runner(nc)
    res = _RUNNER_CACHE[key](in_maps)
    out = np.concatenate([res[k]["out"] for k in range(NCORES)], axis=0)
    return out.astype(np.float32)

